# revision 1
# baseline (speedup 1.0000x reference)
"""Trainium2 Bass kernel for nn_MultiHeadCrossAttention (B=4, T=1024, E=1024, H=16).

Sharding: the computation splits into 8 fully independent shards with zero
cross-core communication: (output stream s, batch b) for s in {1,2}, b in 0..3.
Stream-1 output xo@Wout1 needs K,V from x and Q from y; stream-2 the reverse.
Core c<4 computes stream-1 batch c; core c>=4 computes stream-2 batch c-4.

Per-core kernel (all activations kept transposed, feature-on-partition):
  Q^T = Wq^T.T @ B^T, K^T = Wk^T.T @ A^T   (fp32r matmuls, K=1024)
  V   = A^T.T @ Wv^T                        (natural layout, bf16 store)
  per head pair (2m, 2m+1), row-tiled K=64 matmuls:
    S^T[j,i] = K^T.T @ Q^T;  P^T = exp(S^T/8) on ACT (bf16)
    O'^T = V.T @ P^T (col-tiled M=64 pairs) ; rowsums via M=1 ones-matmuls
    recip = 1/rowsum (DVE); broadcast via K=1 ones-matmul; O^T = O'^T * bcast
  Z^T = Wout^T.T @ O^T  (fp32r, accumulate over head chunks)
Host side pre-transposes/groups weights and activations, and re-transposes
the per-core outputs.
"""

import os
import sys
import time

sys.path.insert(0, "/opt/trn_rl_repo")

import numpy as np
import ml_dtypes
from contextlib import ExitStack

import concourse.bass as bass
import concourse.mybir as mybir
import concourse.tile as tile
from concourse import bacc
from concourse import bass_utils

B, T, E, H = 4, 1024, 1024, 16
D = E // H            # 64
NC = E // 128         # 8 chunks of 128
NIC = T // 512        # 2 free-dim chunks of 512
N_CORES = 8

F32R = mybir.dt.float32r
F32 = mybir.dt.float32
BF16 = mybir.dt.bfloat16
F16 = mybir.dt.float16

_NC_CACHE = {}
LAST_RESULTS = {}

_SELBC = np.zeros((33, 128), np.float32)
_SELBC[0, 0:64] = 1.0
_SELBC[32, 64:128] = 1.0


KPHASE = os.environ.get("KPHASE", "PAZ")
KSKIP = set(os.environ.get("KSKIP", "").split(","))



def _build():
    KREP = int(os.environ.get("KREP", "1"))
    nc = bacc.Bacc("TRN2", target_bir_lowering=False, debug=False,
                   enable_asserts=False, num_devices=N_CORES)
    a_t = nc.dram_tensor("a_t", (E, T), F16, kind="ExternalInput").ap()
    b_t = nc.dram_tensor("b_t", (E, T), F16, kind="ExternalInput").ap()
    wq_t = nc.dram_tensor("wq_t", (E, E), F16, kind="ExternalInput").ap()
    wk_t = nc.dram_tensor("wk_t", (E, E), F16, kind="ExternalInput").ap()
    wv_t = nc.dram_tensor("wv_t", (E, E), F16, kind="ExternalInput").ap()
    wout_t = nc.dram_tensor("wout_t", (E, E), F16, kind="ExternalInput").ap()
    selbc_d = nc.dram_tensor("selbc", (33, 128), F16, kind="ExternalInput").ap()
    z_t = nc.dram_tensor("z_t", (E, T), F32, kind="ExternalOutput").ap()

    with tile.TileContext(nc) as tc, ExitStack() as ctx:
        # long-lived pools
        qkv_pool = ctx.enter_context(tc.tile_pool(name="qkv", bufs=1))
        const_pool = ctx.enter_context(tc.tile_pool(name="const", bufs=1))

        qt = qkv_pool.tile([128, NC, T], F16, tag="qt")
        kt = qkv_pool.tile([128, NC, T], F16, tag="kt")
        v = qkv_pool.tile([128, NC, H * (D + 1)], F16, tag="v")

        for _m in range(NC):
            nc.vector.memset(
                v[:, _m, :].rearrange("p (h x) -> p h x", x=D + 1)[:, :, D:D + 1], 1.0)
        if KSKIP & {"proj"}:
            for m in range(NC):
                nc.vector.memset(qt[:, m, :], 0.25)
                nc.vector.memset(kt[:, m, :], 0.25)
                nc.vector.memset(v[:, m, :], 0.25)
        selbc = const_pool.tile([33, 128], F16, tag="selbc")
        nc.sync.dma_start(selbc[:], selbc_d)

        # ---------------- Phase P: projections ----------------
        for _rep in range(KREP):
            _build_body(nc, tc, ctx, locals())
    nc.compile()
    return nc


def _build_body(nc, tc, ctx, env):
    qt, kt, v = env["qt"], env["kt"], env["v"]
    selbc = env["selbc"]
    a_t, b_t = env["a_t"], env["b_t"]
    wq_t, wk_t, wv_t, wout_t = env["wq_t"], env["wk_t"], env["wv_t"], env["wout_t"]
    z_t = env["z_t"]
    if True:
        with tc.tile_pool(name="acts", bufs=1) as acts, \
             tc.tile_pool(name="pps", bufs=3, space="PSUM") as pps:
            at_sb = acts.tile([128, NC, T], F16, tag="at")
            bt_sb = acts.tile([128, NC, T], F16, tag="bt")
            wv_sb = acts.tile([128, NC, E], F16, tag="wv")
            wq_sb = acts.tile([128, NC, E], F16, tag="wq")
            wk_sb = acts.tile([128, NC, E], F16, tag="wk")
            # issue order matters: Q^T-proj (bt, wq) starts first
            for c in range(NC):
                nc.sync.dma_start(bt_sb[:, c, :], b_t[c * 128:(c + 1) * 128, :])
                nc.sync.dma_start(wq_sb[:, c, :], wq_t[c * 128:(c + 1) * 128, :])
            for c in range(NC):
                nc.sync.dma_start(at_sb[:, c, :], a_t[c * 128:(c + 1) * 128, :])
                nc.sync.dma_start(wk_sb[:, c, :], wk_t[c * 128:(c + 1) * 128, :])
            for c in range(NC):
                nc.sync.dma_start(wv_sb[:, c, :], wv_t[c * 128:(c + 1) * 128, :])

            # Q^T and K^T: out[dh-chunk m][t] = sum_e w[e, dh] * act[e, t]
            # weight column-blocks streamed per m (each block used exactly once)
            for (w_sb, act_sb, out_sb) in (
                () if "proj" in KSKIP else (
                (wq_sb, bt_sb, qt),
                (wk_sb, at_sb, kt),
            )):
                for m in range(NC):
                    ps = pps.tile([128, T], F32, tag="pp")
                    for e in range(NC):
                        for ic in range(NIC):
                            nc.tensor.matmul(
                                ps[:, bass.ts(ic, 512)],
                                w_sb[:, e, bass.ts(m, 128)],
                                act_sb[:, e, bass.ts(ic, 512)],
                                start=(e == 0), stop=(e == NC - 1))
                    nc.scalar.copy(out_sb[:, m, :], ps[:])
            # V natural: out[j-chunk][dv] = sum_e at[e, j] * wv[e, dv]
            for m in range(NC) if "proj" not in KSKIP else ():
                ps = pps.tile([128, T], F32, tag="pp")
                for e in range(NC):
                    for ic in range(NIC):
                        nc.tensor.matmul(
                            ps[:, bass.ts(ic, 512)],
                            at_sb[:, e, bass.ts(m, 128)],
                            wv_sb[:, e, bass.ts(ic, 512)],
                            start=(e == 0), stop=(e == NC - 1))
                nc.scalar.copy(
                    v[:, m, :].rearrange("p (h x) -> p h x", x=D + 1)[:, :, 0:D],
                    ps[:].rearrange("p (h x) -> p h x", x=D))

        if KPHASE == "P":
            with tc.tile_pool(name="zdbg", bufs=2) as zdbgp:
                for cc in range(NC):
                    zdbg = zdbgp.tile([128, T], F32, tag="zdbg")
                    nc.vector.tensor_copy(zdbg[:], qt[:, cc, :])
                    nc.sync.dma_start(z_t[cc * 128:(cc + 1) * 128, :], zdbg[:])
            return

        # ---------------- Phase A: attention per head pair ----------------
        rep_ctx = ctx.enter_context(ExitStack())
        shps = rep_ctx.enter_context(tc.tile_pool(name="shps", bufs=2, space="PSUM"))
        ot_pool = rep_ctx.enter_context(tc.tile_pool(name="ot", bufs=1))
        ot = ot_pool.tile([128, NC, T], F16, tag="ot")
        with tc.tile_pool(name="pt", bufs=4) as ptp, \
             tc.tile_pool(name="nrm", bufs=4) as nrm, \
             tc.tile_pool(name="nrm8", bufs=8) as nrm8, \
             tc.tile_pool(name="ops", bufs=2, space="PSUM") as ops:
            for m in range(NC):
                ptA = ptp.tile([128, NC, T], F16, tag="pt")
                ptB = ptp.tile([128, NC, T], F16, tag="pt")
                if "sexp" in KSKIP:
                    nc.vector.memset(ptA[:], 1.0)
                    nc.vector.memset(ptB[:], 1.0)
                for jc in range(NC) if "sexp" not in KSKIP else ():
                    ps_s = shps.tile([128, T], F32, tag="big")
                    ps_sB = shps.tile([128, T], F32, tag="big")
                    for ic in range(NIC):
                        nc.tensor.matmul(
                            ps_s[:, bass.ts(ic, 512)],
                            kt[0:64, m, bass.ts(jc, 128)],
                            qt[0:64, m, bass.ts(ic, 512)],
                            start=True, stop=True)
                        nc.tensor.matmul(
                            ps_sB[:, bass.ts(ic, 512)],
                            kt[64:128, m, bass.ts(jc, 128)],
                            qt[64:128, m, bass.ts(ic, 512)],
                            start=True, stop=True, tile_position=(64, 0))
                    nc.scalar.activation(ptA[:, jc, :], ps_s[:],
                                         mybir.ActivationFunctionType.Exp, scale=0.125)
                    nc.scalar.activation(ptB[:, jc, :], ps_sB[:],
                                         mybir.ActivationFunctionType.Exp, scale=0.125)

                if "ovr" in KSKIP:
                    nc.vector.memset(ot[:, m, :], 0.25)
                    continue
                ps_oA = ops.tile([65, T], F32, tag="o")
                ps_oB = ops.tile([65, T], F32, tag="o")
                hA, hB = 2 * m, 2 * m + 1
                for jc in range(NC):
                    st = dict(start=(jc == 0), stop=(jc == NC - 1))
                    for ic in range(NIC):
                        s_ic = bass.ts(ic, 512)
                        nc.tensor.matmul(ps_oA[:, s_ic], v[:, jc, bass.ts(hA, D + 1)],
                                         ptA[:, jc, s_ic], **st)
                        nc.tensor.matmul(ps_oB[:, s_ic], v[:, jc, bass.ts(hB, D + 1)],
                                         ptB[:, jc, s_ic], **st)

                recip2 = nrm.tile([33, T], F16, tag="recip2")
                nc.vector.memset(recip2[:], 0.0)
                with nc.allow_low_precision(reason="recip feeds fp16 bc matmul"):
                    nc.vector.reciprocal(recip2[0:1, :], ps_oA[64:65, :])
                    nc.vector.reciprocal(recip2[32:33, :], ps_oB[64:65, :])
                ps_bcA = shps.tile([64, T], F32, tag="big")
                ps_bcB = shps.tile([64, T], F32, tag="big")
                for ic in range(NIC):
                    s_ic = bass.ts(ic, 512)
                    nc.tensor.matmul(ps_bcA[:, s_ic], selbc[:, 0:64], recip2[:, s_ic],
                                     start=True, stop=True)
                    nc.tensor.matmul(ps_bcB[:, s_ic], selbc[:, 64:128], recip2[:, s_ic],
                                     start=True, stop=True)
                bcA = nrm.tile([64, T], F32, tag="bcA")
                bcB = nrm.tile([64, T], F32, tag="bcB")
                nc.scalar.copy(bcA[:], ps_bcA[:])
                nc.scalar.copy(bcB[:], ps_bcB[:])
                with nc.allow_low_precision(reason="O^T fp16 feeds fp16 out-proj"):
                    nc.vector.tensor_mul(ot[0:64, m, :], ps_oA[0:64, :], bcA[:])
                    nc.vector.tensor_mul(ot[64:128, m, :], ps_oB[0:64, :], bcB[:])

        if KPHASE == "PA":
            with tc.tile_pool(name="zdbg", bufs=2) as zdbgp:
                for cc in range(NC):
                    zdbg = zdbgp.tile([128, T], F32, tag="zdbg")
                    nc.vector.tensor_copy(zdbg[:], ot[:, cc, :])
                    nc.sync.dma_start(z_t[cc * 128:(cc + 1) * 128, :], zdbg[:])
            return

        # ---------------- Phase Z: out-projection ----------------
        with tc.tile_pool(name="wout", bufs=1) as woutp, \
             tc.tile_pool(name="zsb", bufs=2) as zsbp, \
             tc.tile_pool(name="zps", bufs=2, space="PSUM") as zps:
            wo = woutp.tile([128, NC, E], F16, tag="wo")
            for c in range(NC):
                nc.sync.dma_start(wo[:, c, :], wout_t[c * 128:(c + 1) * 128, :])
            for cc in range(NC):
                ps = zps.tile([128, T], F32, tag="z")
                for m in range(NC):
                    for ic in range(NIC):
                        nc.tensor.matmul(
                            ps[:, bass.ts(ic, 512)],
                            wo[:, m, bass.ts(cc, 128)],
                            ot[:, m, bass.ts(ic, 512)],
                            start=(m == 0), stop=(m == NC - 1))
                zsb = zsbp.tile([128, T], F32, tag="zsb")
                nc.scalar.copy(zsb[:], ps[:])
                nc.sync.dma_start(z_t[cc * 128:(cc + 1) * 128, :], zsb[:])
        rep_ctx.close()


def _group_w(wqkv, k):
    """Rows of Wqkv (3E, E) for q/k/v (k=0/1/2), grouped head-major.

    Row index layout: r = di*(3H) + k*H + h  ->  grouped[h*D+di, :].
    """
    w = np.asarray(wqkv, dtype=np.float32).reshape(D, 3, H, E)[:, k]   # [di, h, e]
    return np.ascontiguousarray(w.transpose(1, 0, 2).reshape(E, E))    # [h*D+di, e]


def kernel(x, y, Wqkv1, Wqkv2, Wout1, Wout2):
    x = np.asarray(x, dtype=np.float32)
    y = np.asarray(y, dtype=np.float32)

    if "nc" not in _NC_CACHE:
        _NC_CACHE["nc"] = _build()
    nc = _NC_CACHE["nc"]

    # weight prep (host): grouped + transposed (fp16 on-device dtype)
    wq1_t = np.ascontiguousarray(_group_w(Wqkv1, 0).T)
    wk1_t = np.ascontiguousarray(_group_w(Wqkv1, 1).T)
    wv1_t = np.ascontiguousarray(_group_w(Wqkv1, 2).T)
    wq2_t = np.ascontiguousarray(_group_w(Wqkv2, 0).T)
    wk2_t = np.ascontiguousarray(_group_w(Wqkv2, 1).T)
    wv2_t = np.ascontiguousarray(_group_w(Wqkv2, 2).T)
    wout1_t = np.ascontiguousarray(np.asarray(Wout1, dtype=np.float32).T)
    wout2_t = np.ascontiguousarray(np.asarray(Wout2, dtype=np.float32).T)

    in_maps = []
    for c in range(N_CORES):
        s, b = divmod(c, B)
        if s == 0:
            # stream-1 output: K,V from x via Wqkv1; Q from y via Wqkv2
            a_t, b_t = x[b].T, y[b].T
            wq, wk, wv, wo = wq2_t, wk1_t, wv1_t, wout1_t
        else:
            a_t, b_t = y[b].T, x[b].T
            wq, wk, wv, wo = wq1_t, wk2_t, wv2_t, wout2_t
        in_maps.append({
            "a_t": np.ascontiguousarray(a_t).astype(np.float16),
            "b_t": np.ascontiguousarray(b_t).astype(np.float16),
            "wq_t": wq.astype(np.float16), "wk_t": wk.astype(np.float16),
            "wv_t": wv.astype(np.float16), "wout_t": wo.astype(np.float16),
            "selbc": _SELBC.astype(np.float16),
        })

    trace = os.environ.get("BASS_KERNEL_TRACE", "0") == "1"
    if trace:
        try:
            from antenv.axon_hooks import get_axon_ntff_profile_hook  # noqa: F401
        except ImportError:
            trace = False
    ncores = int(os.environ.get("KCORES", str(N_CORES)))
    r = bass_utils.run_bass_kernel_spmd(nc, in_maps[:ncores], core_ids=list(range(ncores)),
                                        trace=trace)
    LAST_RESULTS["exec_time_ns"] = r.exec_time_ns
    LAST_RESULTS["profile_json"] = r.profile_json

    out1 = np.stack([r.results[b]["z_t"].T for b in range(B)]).astype(np.float32)
    out2 = np.stack([r.results[B + b]["z_t"].T for b in range(B)]).astype(np.float32)
    return out1, out2



# revision 20
# speedup vs baseline: 1.3243x; 1.3243x over previous
"""Trainium2 Bass kernel for nn_MultiHeadCrossAttention (B=4, T=1024, E=1024, H=16).

Sharding: 8 fully independent shards (output stream s, batch b), zero
cross-core communication. Stream-1 output xo@Wout1 needs K,V from x and Q
from y; stream-2 the reverse. Core c<4 computes stream-1 batch c; core c>=4
stream-2 batch c-4.

Per-core design: one flat software-pipelined stream over 64 (head-pair m,
key-chunk jc) units, paced by ScalarE exp (the irreducible ~147us of
softmax exponentials). Everything else hides inside that window so the
tensor engine never idles >3.4us (HAM stays at full clock):
  unit (m, jc): S^T = K^T.T @ Q^T (row-paired K=64 MMs, [128,T] PSUM);
                P^T = exp(S^T/8) (2 ACTs); O' ic0-half accumulates
                ([65,512] PSUM, ones-row gives rowsums for free).
  injections (relative to unit stream):
    m=0 units carry V-projection for chunk jc just-in-time.
    (m,0)/(m,1): O' ic1-half of m-1 (A then B, staggered for PSUM slots);
    (m,1)/(m,2): normalize m-1 (reciprocal_approx_fast on free-dim rowsums,
                 K=1 ones-matmul broadcast, DVE mul into O^T f16);
    (m,4)/(m,6): Q^T/K^T projections for chunk m+1 (weights streamed from
                 HBM in host-swizzled contiguous blocks).
  tail: flush m=7, then Z^T = Wout^T.T @ O^T per 128-row chunk (m=7 term
  accumulated last so Z overlaps the flush).
PSUM budget (8 banks): S units 2x[128,1024]=4, O' accums 3x[65,512]=3,
proj/bc scratch 1x[128,512]=1.
Host pre-transposes/groups weights and activations; re-transposes outputs.
"""

import os
import sys

sys.path.insert(0, "/opt/trn_rl_repo")

import numpy as np
from contextlib import ExitStack

import concourse.bass as bass
import concourse.mybir as mybir
import concourse.tile as tile
from concourse import bacc
from concourse import bass_utils

B, T, E, H = 4, 1024, 1024, 16
D = E // H            # 64
NC = E // 128         # 8 chunks of 128
NIC = T // 512        # 2 free-dim chunks of 512
N_CORES = 8

F32 = mybir.dt.float32
F16 = mybir.dt.float16
EXP = mybir.ActivationFunctionType.Exp

_NC_CACHE = {}
LAST_RESULTS = {}


def _build():
    nc = bacc.Bacc("TRN2", target_bir_lowering=False, debug=False,
                   enable_asserts=False, num_devices=N_CORES)
    a_t = nc.dram_tensor("a_t", (E, T), F16, kind="ExternalInput").ap()
    b_t = nc.dram_tensor("b_t", (E, T), F16, kind="ExternalInput").ap()
    wq_t = nc.dram_tensor("wq_t", (E, E), F16, kind="ExternalInput").ap()
    wk_t = nc.dram_tensor("wk_t", (E, E), F16, kind="ExternalInput").ap()
    wv_t = nc.dram_tensor("wv_t", (E, E), F16, kind="ExternalInput").ap()
    wout_t = nc.dram_tensor("wout_t", (E, E), F16, kind="ExternalInput").ap()
    z_t = nc.dram_tensor("z_t", (E, T), F32, kind="ExternalOutput").ap()

    # wq_t/wk_t/wout_t are host-swizzled: block m of w^T is the contiguous
    # [128, 1024] slice rows m*128..(m+1)*128, laid out [p, e*128+c].
    def wblock(w, m):
        return w[m * 128:(m + 1) * 128, :]

    with tile.TileContext(nc) as tc, ExitStack() as ctx:
        persist = ctx.enter_context(tc.tile_pool(name="persist", bufs=1))
        qt = persist.tile([128, NC, T], F16, tag="qt")
        kt = persist.tile([128, NC, T], F16, tag="kt")
        v = persist.tile([128, NC, H * (D + 1)], F16, tag="v")
        at_sb = persist.tile([128, NC, T], F16, tag="at")
        bt_sb = persist.tile([128, NC, T], F16, tag="bt")
        ot = persist.tile([128, NC, T], F16, tag="ot")      # normalized O^T
        ones_t = persist.tile([1, 128], F32, tag="ones")
        nc.vector.memset(ones_t[:], 1.0)

        # DMA order matters: bt + first weight chunks first so Q0/K0 start
        # early; at/wv interleaved so V-proj can chase the transfers.
        wch = ctx.enter_context(tc.tile_pool(name="wch", bufs=2))
        wq0 = wch.tile([128, NC * 128], F16, tag="w", name="wq0")
        wk0 = wch.tile([128, NC * 128], F16, tag="w", name="wk0")
        nc.sync.dma_start(wq0[:], wblock(wq_t, 0))
        nc.sync.dma_start(wk0[:], wblock(wk_t, 0))
        for c in range(NC):
            nc.sync.dma_start(bt_sb[:, c, :], b_t[c * 128:(c + 1) * 128, :])
            nc.sync.dma_start(at_sb[:, c, :], a_t[c * 128:(c + 1) * 128, :])

        wvp = ctx.enter_context(tc.tile_pool(name="wv", bufs=1))
        wv_sb = wvp.tile([128, NC, E], F16, tag="wv")
        for c in range(NC):
            nc.sync.dma_start(wv_sb[:, c, :], wv_t[c * 128:(c + 1) * 128, :])

        # ones column per head in V (col D within each D+1 group) -> rowsums
        for m in range(NC):
            nc.vector.memset(
                v[:, m, :].rearrange("p (h x) -> p h x", x=D + 1)[:, :, D:D + 1], 1.0)

        ptp = ctx.enter_context(tc.tile_pool(name="pt", bufs=4))
        rsp = ctx.enter_context(tc.tile_pool(name="rsp", bufs=1))
        bcp = ctx.enter_context(tc.tile_pool(name="bcp", bufs=2))
        bigp = ctx.enter_context(tc.tile_pool(name="bigp", bufs=2, space="PSUM"))
        opool = ctx.enter_context(tc.tile_pool(name="op", bufs=4, space="PSUM"))

        def proj_chunk(w_sb, act_sb, out_sb, m, on_act):
            """One [128, T] projection chunk (contract over e)."""
            ps = bigp.tile([128, T], F32, tag="big", name=f"pj{m}")
            for ic in range(NIC):
                for e in range(NC):
                    nc.tensor.matmul(
                        ps[:, bass.ts(ic, 512)], w_sb[:, bass.ts(e, 128)],
                        act_sb[:, e, bass.ts(ic, 512)],
                        start=(e == 0), stop=(e == NC - 1))
            if on_act:
                nc.scalar.copy(out_sb[:, m, :], ps[:])
            else:
                nc.vector.tensor_copy(out_sb[:, m, :], ps[:])

        def vproj_chunk(jc):
            """V chunk jc (natural layout), copy strided into (h, D+1)."""
            ps = bigp.tile([128, T], F32, tag="big", name=f"vps{jc}")
            for ic in range(NIC):
                for e in range(NC):
                    nc.tensor.matmul(
                        ps[:, bass.ts(ic, 512)], at_sb[:, e, bass.ts(jc, 128)],
                        wv_sb[:, e, bass.ts(ic, 512)],
                        start=(e == 0), stop=(e == NC - 1))
            nc.vector.tensor_copy(
                v[:, jc, :].rearrange("p (h x) -> p h x", x=D + 1)[:, :, 0:D],
                ps[:].rearrange("p (h x) -> p h x", x=D))

        proj_chunk(wq0, bt_sb, qt, 0, on_act=True)
        proj_chunk(wk0, at_sb, kt, 0, on_act=True)

        # per-m pipeline state
        state = {}

        def normalize_half(m, ic, accA, accB, joint):
            """rowsum -> 1/r -> broadcast -> O^T = O' * (1/r), for one ic half.

            joint=True: both heads at once (acc rowsums both available).
            """
            hs = [(0, accA), (1, accB)] if joint else [(0, accA), (1, accB)]
            rsf = rsp.tile([1, 2, 512], F32, tag="rsf", name=f"rsf{m}{ic}")
            rsr = rsp.tile([1, 2, 512], F32, tag="rsr", name=f"rsr{m}{ic}")
            for h, acc in hs:
                nc.vector.tensor_copy(rsf[0:1, h, :], acc[64:65, :])
            nc.vector.reciprocal_approx_fast(
                rsr[0:1, :, :].rearrange("p a b -> p (a b)"),
                rsf[0:1, :, :].rearrange("p a b -> p (a b)"))
            bc = bigp.tile([128, T], F32, tag="big", name=f"bc{m}{ic}")
            nc.tensor.matmul(bc[0:64, 0:512], ones_t[0:1, 0:64],
                             rsr[0:1, 0, :], start=True, stop=True)
            nc.tensor.matmul(bc[64:128, 0:512], ones_t[0:1, 0:64],
                             rsr[0:1, 1, :], start=True, stop=True,
                             tile_position=(0, 64))
            bcs = bcp.tile([128, 512], F32, tag="bcs", name=f"bcs{m}{ic}")
            nc.vector.tensor_copy(bcs[:], bc[:, 0:512])
            s_ic = bass.ts(ic, 512)
            with nc.allow_low_precision(reason="O^T f16 feeds f16 out-proj"):
                nc.vector.tensor_mul(ot[0:64, m, s_ic], accA[0:64, :], bcs[0:64, :])
                nc.vector.tensor_mul(ot[64:128, m, s_ic], accB[0:64, :],
                                     bcs[64:128, :])

        def o_ic1(m, head):
            """O' ic1-half accumulation for one head of pair m."""
            st_ = state[m]
            acc = opool.tile([65, 512], F32, tag="o", name=f"oic1{m}{head}")
            pt = st_["ptA"] if head == 0 else st_["ptB"]
            hh = 2 * m + head
            for jc in range(NC):
                nc.tensor.matmul(acc[:], v[:, jc, bass.ts(hh, D + 1)],
                                 pt[:, jc, 512:1024],
                                 start=(jc == 0), stop=(jc == NC - 1))
            st_[f"acc1_{head}"] = acc

        def norm_ic1(m):
            st_ = state[m]
            normalize_half(m, 1, st_["acc1_0"], st_["acc1_1"], joint=True)

        # ---------------- the 64-unit stream ----------------
        for u in range(NC * NC):
            m, jc = divmod(u, NC)
            hA, hB = 2 * m, 2 * m + 1
            if jc == 0:
                state[m] = {
                    "ptA": ptp.tile([128, NC, T], F16, tag="pt", name=f"ptA{m}"),
                    "ptB": ptp.tile([128, NC, T], F16, tag="pt", name=f"ptB{m}"),
                }
            st_ = state[m]
            ptA, ptB = st_["ptA"], st_["ptB"]

            ps_s = bigp.tile([128, T], F32, tag="big", name=f"sA{u}")
            ps_sB = bigp.tile([128, T], F32, tag="big", name=f"sB{u}")
            for ic in range(NIC):
                nc.tensor.matmul(
                    ps_s[:, bass.ts(ic, 512)],
                    kt[0:64, m, bass.ts(jc, 128)],
                    qt[0:64, m, bass.ts(ic, 512)],
                    start=True, stop=True)
                nc.tensor.matmul(
                    ps_sB[:, bass.ts(ic, 512)],
                    kt[64:128, m, bass.ts(jc, 128)],
                    qt[64:128, m, bass.ts(ic, 512)],
                    start=True, stop=True, tile_position=(64, 0))
            nc.scalar.activation(ptA[:, jc, :], ps_s[:], EXP, scale=0.125)
            nc.scalar.activation(ptB[:, jc, :], ps_sB[:], EXP, scale=0.125)

            if m == 0:
                vproj_chunk(jc)   # just-in-time V for O' below

            # ---- injections (pipelined work of m-1 / m+1); emitted BEFORE
            # this unit's O' matmuls so normalize's bc matmuls precede any
            # PE instruction that waits on the slots its muls release ----
            if jc == 0 and m >= 1:
                normalize_half(m - 1, 0, state[m - 1]["acc0_0"],
                               state[m - 1]["acc0_1"], joint=True)
                o_ic1(m - 1, 0)
            elif jc == 1 and m >= 1:
                o_ic1(m - 1, 1)
            elif jc == 2 and m >= 1:
                norm_ic1(m - 1)
                del state[m - 1]
            elif jc == 4 and m + 1 < NC:
                wq_sb = wch.tile([128, NC * 128], F16, tag="w", name=f"wq{m+1}")
                nc.sync.dma_start(wq_sb[:], wblock(wq_t, m + 1))
                proj_chunk(wq_sb, bt_sb, qt, m + 1, on_act=False)
            elif jc == 6 and m + 1 < NC:
                wk_sb = wch.tile([128, NC * 128], F16, tag="w", name=f"wk{m+1}")
                nc.sync.dma_start(wk_sb[:], wblock(wk_t, m + 1))
                proj_chunk(wk_sb, at_sb, kt, m + 1, on_act=False)

            # O' ic0-half accumulation, start deferred to jc==3 so the slot
            # wait lands after normalize(m-1) has released its accumulators
            if jc == 3:
                st_["acc0_0"] = opool.tile([65, 512], F32, tag="o", name=f"o0A{m}")
                st_["acc0_1"] = opool.tile([65, 512], F32, tag="o", name=f"o0B{m}")
                for j2 in range(4):
                    stf = dict(start=(j2 == 0), stop=False)
                    nc.tensor.matmul(st_["acc0_0"][:], v[:, j2, bass.ts(hA, D + 1)],
                                     ptA[:, j2, 0:512], **stf)
                    nc.tensor.matmul(st_["acc0_1"][:], v[:, j2, bass.ts(hB, D + 1)],
                                     ptB[:, j2, 0:512], **stf)
            elif jc > 3:
                stf = dict(start=False, stop=(jc == NC - 1))
                nc.tensor.matmul(st_["acc0_0"][:], v[:, jc, bass.ts(hA, D + 1)],
                                 ptA[:, jc, 0:512], **stf)
                nc.tensor.matmul(st_["acc0_1"][:], v[:, jc, bass.ts(hB, D + 1)],
                                 ptB[:, jc, 0:512], **stf)

        # ---------------- flush m=7 + out-projection ----------------
        m = NC - 1
        o_ic1(m, 0)
        normalize_half(m, 0, state[m]["acc0_0"], state[m]["acc0_1"], joint=True)
        o_ic1(m, 1)
        norm_ic1(m)

        with tc.tile_pool(name="woch", bufs=2) as wochp, \
             tc.tile_pool(name="zsb", bufs=1) as zsbp:
            for cc in range(NC):
                wo_sb = wochp.tile([128, NC * 128], F16, tag="wo", name=f"wo{cc}")
                nc.sync.dma_start(wo_sb[:], wblock(wout_t, cc))
                ps = bigp.tile([128, T], F32, tag="big", name=f"z{cc}")
                # m=7 term last: its O^T lands during the flush above
                mm_order = list(range(NC - 1)) + [NC - 1]
                for i, mm in enumerate(mm_order):
                    for ic in range(NIC):
                        nc.tensor.matmul(
                            ps[:, bass.ts(ic, 512)],
                            wo_sb[:, bass.ts(mm, 128)],
                            ot[:, mm, bass.ts(ic, 512)],
                            start=(i == 0), stop=(i == NC - 1))
                zsb = zsbp.tile([128, T], F32, tag="zsb", name=f"zsb{cc}")
                nc.vector.tensor_copy(zsb[:], ps[:])
                nc.sync.dma_start(z_t[cc * 128:(cc + 1) * 128, :], zsb[:])
    nc.compile()
    return nc


def _group_w(wqkv, k):
    """Rows of Wqkv (3E, E) for q/k/v (k=0/1/2), grouped head-major.

    Row index layout: r = di*(3H) + k*H + h  ->  grouped[h*D+di, :].
    """
    w = np.asarray(wqkv, dtype=np.float32).reshape(D, 3, H, E)[:, k]   # [di, h, e]
    return np.ascontiguousarray(w.transpose(1, 0, 2).reshape(E, E))    # [h*D+di, e]


def _stream_layout(w_t):
    """Swizzle w^T [e*128+p, m*128+c] -> [m*128+p, e*128+c] so the device can
    stream output-block m as one contiguous [128, 1024] DMA."""
    a = np.asarray(w_t).reshape(NC, 128, NC, 128)
    return np.ascontiguousarray(a.transpose(2, 1, 0, 3).reshape(E, E))


def kernel(x, y, Wqkv1, Wqkv2, Wout1, Wout2):
    x = np.asarray(x, dtype=np.float32)
    y = np.asarray(y, dtype=np.float32)

    if "nc" not in _NC_CACHE:
        _NC_CACHE["nc"] = _build()
    nc = _NC_CACHE["nc"]

    # weight prep (host): grouped + transposed (f16 on-device dtype)
    wq1_t = np.ascontiguousarray(_group_w(Wqkv1, 0).T)
    wk1_t = np.ascontiguousarray(_group_w(Wqkv1, 1).T)
    wv1_t = np.ascontiguousarray(_group_w(Wqkv1, 2).T)
    wq2_t = np.ascontiguousarray(_group_w(Wqkv2, 0).T)
    wk2_t = np.ascontiguousarray(_group_w(Wqkv2, 1).T)
    wv2_t = np.ascontiguousarray(_group_w(Wqkv2, 2).T)
    wout1_t = np.ascontiguousarray(np.asarray(Wout1, dtype=np.float32).T)
    wout2_t = np.ascontiguousarray(np.asarray(Wout2, dtype=np.float32).T)

    in_maps = []
    for c in range(N_CORES):
        s, b = divmod(c, B)
        if s == 0:
            # stream-1 output: K,V from x via Wqkv1; Q from y via Wqkv2
            a_t, b_t = x[b].T, y[b].T
            wq, wk, wv, wo = wq2_t, wk1_t, wv1_t, wout1_t
        else:
            a_t, b_t = y[b].T, x[b].T
            wq, wk, wv, wo = wq1_t, wk2_t, wv2_t, wout2_t
        in_maps.append({
            "a_t": np.ascontiguousarray(a_t).astype(np.float16),
            "b_t": np.ascontiguousarray(b_t).astype(np.float16),
            "wq_t": _stream_layout(wq).astype(np.float16),
            "wk_t": _stream_layout(wk).astype(np.float16),
            "wv_t": wv.astype(np.float16),
            "wout_t": _stream_layout(wo).astype(np.float16),
        })

    trace = os.environ.get("BASS_KERNEL_TRACE", "0") == "1"
    if trace:
        try:
            from antenv.axon_hooks import get_axon_ntff_profile_hook  # noqa: F401
        except ImportError:
            trace = False
    ncores = int(os.environ.get("KCORES", str(N_CORES)))
    r = bass_utils.run_bass_kernel_spmd(nc, in_maps[:ncores], core_ids=list(range(ncores)),
                                        trace=trace)
    LAST_RESULTS["exec_time_ns"] = r.exec_time_ns
    LAST_RESULTS["profile_json"] = r.profile_json

    out1 = np.stack([r.results[b]["z_t"].T for b in range(B)]).astype(np.float32)
    out2 = np.stack([r.results[B + b]["z_t"].T for b in range(B)]).astype(np.float32)
    return out1, out2


# revision 25
# speedup vs baseline: 1.3406x; 1.0123x over previous
"""Trainium2 Bass kernel for nn_MultiHeadCrossAttention (B=4, T=1024, E=1024, H=16).

Sharding: 8 fully independent shards (output stream s, batch b), zero
cross-core communication. Stream-1 output xo@Wout1 needs K,V from x and Q
from y; stream-2 the reverse. Core c<4 computes stream-1 batch c; core c>=4
stream-2 batch c-4.

Per-core design: one flat software-pipelined stream over 64 (head-pair m,
key-chunk jc) units, paced by ScalarE exp (the irreducible ~147us of
softmax exponentials). Everything else hides inside that window so the
tensor engine never idles >3.4us (HAM stays at full clock):
  unit (m, jc): S^T = K^T.T @ Q^T (row-paired K=64 MMs, [128,T] PSUM);
                P^T = exp(S^T/8) (2 ACTs); O' ic0-half accumulates
                ([65,512] PSUM, ones-row gives rowsums for free).
  injections (relative to unit stream):
    m=0 units carry V-projection for chunk jc just-in-time.
    (m,0)/(m,1): O' ic1-half of m-1 (A then B, staggered for PSUM slots);
    (m,1)/(m,2): normalize m-1 (reciprocal_approx_fast on free-dim rowsums,
                 K=1 ones-matmul broadcast, DVE mul into O^T f16);
    (m,4)/(m,6): Q^T/K^T projections for chunk m+1 (weights streamed from
                 HBM in host-swizzled contiguous blocks).
  tail: flush m=7, then Z^T = Wout^T.T @ O^T per 128-row chunk (m=7 term
  accumulated last so Z overlaps the flush).
PSUM budget (8 banks): S units 2x[128,1024]=4, O' accums 3x[65,512]=3,
proj/bc scratch 1x[128,512]=1.
Host pre-transposes/groups weights and activations; re-transposes outputs.
"""

import os
import sys

sys.path.insert(0, "/opt/trn_rl_repo")

import numpy as np
from contextlib import ExitStack

import concourse.bass as bass
import concourse.mybir as mybir
import concourse.tile as tile
from concourse import bacc
from concourse import bass_utils

B, T, E, H = 4, 1024, 1024, 16
D = E // H            # 64
NC = E // 128         # 8 chunks of 128
NIC = T // 512        # 2 free-dim chunks of 512
N_CORES = 8

F32 = mybir.dt.float32
F16 = mybir.dt.float16
EXP = mybir.ActivationFunctionType.Exp

_NC_CACHE = {}
LAST_RESULTS = {}


def _build():
    nc = bacc.Bacc("TRN2", target_bir_lowering=False, debug=False,
                   enable_asserts=False, num_devices=N_CORES)
    a_t = nc.dram_tensor("a_t", (E, T), F16, kind="ExternalInput").ap()
    b_t = nc.dram_tensor("b_t", (E, T), F16, kind="ExternalInput").ap()
    wq_t = nc.dram_tensor("wq_t", (E, E), F16, kind="ExternalInput").ap()
    wk_t = nc.dram_tensor("wk_t", (E, E), F16, kind="ExternalInput").ap()
    wv_t = nc.dram_tensor("wv_t", (E, E), F16, kind="ExternalInput").ap()
    wout_t = nc.dram_tensor("wout_t", (E, E), F16, kind="ExternalInput").ap()
    z_t = nc.dram_tensor("z_t", (E, T), F32, kind="ExternalOutput").ap()

    # wq_t/wk_t/wout_t are host-swizzled: block m of w^T is the contiguous
    # [128, 1024] slice rows m*128..(m+1)*128, laid out [p, e*128+c].
    def wblock(w, m):
        return w[m * 128:(m + 1) * 128, :]

    with tile.TileContext(nc) as tc, ExitStack() as ctx:
        persist = ctx.enter_context(tc.tile_pool(name="persist", bufs=1))
        qt = persist.tile([128, NC, T], F16, tag="qt")
        kt = persist.tile([128, NC, T], F16, tag="kt")
        v = persist.tile([128, NC, H * (D + 1)], F16, tag="v")
        at_sb = persist.tile([128, NC, T], F16, tag="at")
        bt_sb = persist.tile([128, NC, T], F16, tag="bt")
        ot = persist.tile([128, NC, T], F16, tag="ot")      # normalized O^T
        ones_t = persist.tile([1, 128], F16, tag="ones")
        nc.vector.memset(ones_t[:], 1.0)

        # DMA order matters: bt + first weight chunks first so Q0/K0 start
        # early; at/wv interleaved so V-proj can chase the transfers.
        wch = ctx.enter_context(tc.tile_pool(name="wch", bufs=2))
        wq0 = wch.tile([128, NC * 128], F16, tag="w", name="wq0")
        wk0 = wch.tile([128, NC * 128], F16, tag="w", name="wk0")
        nc.sync.dma_start(wq0[:], wblock(wq_t, 0))
        nc.sync.dma_start(wk0[:], wblock(wk_t, 0))
        for c in range(NC):
            nc.sync.dma_start(bt_sb[:, c, :], b_t[c * 128:(c + 1) * 128, :])
            nc.sync.dma_start(at_sb[:, c, :], a_t[c * 128:(c + 1) * 128, :])

        wvp = ctx.enter_context(tc.tile_pool(name="wv", bufs=1))
        wv_sb = wvp.tile([128, NC, E], F16, tag="wv")
        for c in range(NC):
            nc.sync.dma_start(wv_sb[:, c, :], wv_t[c * 128:(c + 1) * 128, :])

        # ones column per head in V (col D within each D+1 group) -> rowsums
        for m in range(NC):
            nc.vector.memset(
                v[:, m, :].rearrange("p (h x) -> p h x", x=D + 1)[:, :, D:D + 1], 1.0)

        ptp = ctx.enter_context(tc.tile_pool(name="pt", bufs=3))
        rsp = ctx.enter_context(tc.tile_pool(name="rsp", bufs=1))
        bcp = ctx.enter_context(tc.tile_pool(name="bcp", bufs=1))
        bigp = ctx.enter_context(tc.tile_pool(name="bigp", bufs=2, space="PSUM"))
        opool = ctx.enter_context(tc.tile_pool(name="op", bufs=4, space="PSUM"))

        def proj_chunk(w_sb, act_sb, out_sb, m, on_act):
            """One [128, T] projection chunk (contract over e)."""
            ps = bigp.tile([128, T], F32, tag="big", name=f"pj{m}")
            for e in range(NC):
                for ic in range(NIC):
                    nc.tensor.matmul(
                        ps[:, bass.ts(ic, 512)], w_sb[:, bass.ts(e, 128)],
                        act_sb[:, e, bass.ts(ic, 512)],
                        start=(e == 0), stop=(e == NC - 1))
            if on_act:
                nc.scalar.copy(out_sb[:, m, :], ps[:])
            else:
                nc.vector.tensor_copy(out_sb[:, m, :], ps[:])

        def vproj_chunk(jc):
            """V chunk jc (natural layout), copy strided into (h, D+1)."""
            ps = bigp.tile([128, T], F32, tag="big", name=f"vps{jc}")
            for e in range(NC):
                for ic in range(NIC):
                    nc.tensor.matmul(
                        ps[:, bass.ts(ic, 512)], at_sb[:, e, bass.ts(jc, 128)],
                        wv_sb[:, e, bass.ts(ic, 512)],
                        start=(e == 0), stop=(e == NC - 1))
            nc.vector.tensor_copy(
                v[:, jc, :].rearrange("p (h x) -> p h x", x=D + 1)[:, :, 0:D],
                ps[:].rearrange("p (h x) -> p h x", x=D))

        proj_chunk(wq0, bt_sb, qt, 0, on_act=True)
        proj_chunk(wk0, at_sb, kt, 0, on_act=True)

        # per-m pipeline state
        state = {}

        def normalize_m(m):
            """rowsums -> 1/r -> broadcast -> O^T = O' * (1/r), both ic halves.

            Frees all four of m's O'-accumulator PSUM tiles. Its bc matmuls
            are emitted before any PE instruction that waits on those slots.
            """
            st_ = state[m]
            rsf = rsp.tile([1, 2, NIC, 512], F32, tag="rsf", name=f"rsf{m}")
            rsr = rsp.tile([1, 2, NIC, 512], F32, tag="rsr", name=f"rsr{m}")
            for h in range(2):
                for ic in range(NIC):
                    nc.vector.tensor_copy(rsf[0:1, h, ic, :],
                                          st_[f"acc{ic}_{h}"][64:65, :])
            nc.vector.reciprocal_approx_fast(
                rsr[0:1].rearrange("p a b c -> p (a b c)"),
                rsf[0:1].rearrange("p a b c -> p (a b c)"))
            rsr16 = rsp.tile([1, 2, NIC, 512], F16, tag="rsr16", name=f"rsr16_{m}")
            with nc.allow_low_precision(reason="1/r feeds f16 bc matmul"):
                nc.vector.tensor_copy(
                    rsr16[0:1].rearrange("p a b c -> p (a b c)"),
                    rsr[0:1].rearrange("p a b c -> p (a b c)"))
            bc = bigp.tile([128, T], F32, tag="big", name=f"bc{m}")
            for ic in range(NIC):
                nc.tensor.matmul(bc[0:64, bass.ts(ic, 512)], ones_t[0:1, 0:64],
                                 rsr16[0:1, 0, ic, :], start=True, stop=True)
            for ic in range(NIC):
                nc.tensor.matmul(bc[64:128, bass.ts(ic, 512)], ones_t[0:1, 0:64],
                                 rsr16[0:1, 1, ic, :], start=True, stop=True,
                                 tile_position=(0, 64))
            bcs = bcp.tile([128, T], F32, tag="bcs", name=f"bcs{m}")
            nc.vector.tensor_copy(bcs[:], bc[:])
            with nc.allow_low_precision(reason="O^T f16 feeds f16 out-proj"):
                for ic in range(NIC):
                    s_ic = bass.ts(ic, 512)
                    nc.vector.tensor_mul(ot[0:64, m, s_ic],
                                         st_[f"acc{ic}_0"][0:64, :], bcs[0:64, s_ic])
                    nc.vector.tensor_mul(ot[64:128, m, s_ic],
                                         st_[f"acc{ic}_1"][0:64, :],
                                         bcs[64:128, s_ic])

        # ---------------- the 64-unit stream ----------------
        for u in range(NC * NC):
            m, jc = divmod(u, NC)
            hA, hB = 2 * m, 2 * m + 1
            if jc == 0:
                state[m] = {
                    "ptA": ptp.tile([128, NC, T], F16, tag="pt", name=f"ptA{m}"),
                    "ptB": ptp.tile([128, NC, T], F16, tag="pt", name=f"ptB{m}"),
                }
            st_ = state[m]
            ptA, ptB = st_["ptA"], st_["ptB"]

            ps_s = bigp.tile([128, T], F32, tag="big", name=f"sA{u}")
            ps_sB = bigp.tile([128, T], F32, tag="big", name=f"sB{u}")
            for ic in range(NIC):
                nc.tensor.matmul(
                    ps_s[:, bass.ts(ic, 512)],
                    kt[0:64, m, bass.ts(jc, 128)],
                    qt[0:64, m, bass.ts(ic, 512)],
                    start=True, stop=True)
            for ic in range(NIC):
                nc.tensor.matmul(
                    ps_sB[:, bass.ts(ic, 512)],
                    kt[64:128, m, bass.ts(jc, 128)],
                    qt[64:128, m, bass.ts(ic, 512)],
                    start=True, stop=True, tile_position=(64, 0))
            nc.scalar.activation(ptA[:, jc, :], ps_s[:], EXP, scale=0.125)
            nc.scalar.activation(ptB[:, jc, :], ps_sB[:], EXP, scale=0.125)

            if m == 0:
                vproj_chunk(jc)   # just-in-time V for O' below

            # ---- injections (pipelined work of m-1 / m+1); emitted BEFORE
            # this unit's O' matmuls so normalize's bc matmuls precede any
            # PE instruction that waits on the slots its muls release ----
            if jc == 0 and m >= 1:
                normalize_m(m - 1)
                del state[m - 1]
            elif jc == 4 and m + 1 < NC:
                wq_sb = wch.tile([128, NC * 128], F16, tag="w", name=f"wq{m+1}")
                nc.sync.dma_start(wq_sb[:], wblock(wq_t, m + 1))
                proj_chunk(wq_sb, bt_sb, qt, m + 1, on_act=False)
            elif jc == 6 and m + 1 < NC:
                wk_sb = wch.tile([128, NC * 128], F16, tag="w", name=f"wk{m+1}")
                nc.sync.dma_start(wk_sb[:], wblock(wk_t, m + 1))
                proj_chunk(wk_sb, at_sb, kt, m + 1, on_act=False)

            # O' accumulation (both ic halves, one weight load per 2 MMs);
            # start deferred to jc==3 so the slot wait lands after
            # normalize(m-1) has released its four accumulators
            def o_mms(j2, first):
                stf = dict(start=first, stop=(j2 == NC - 1))
                for ic in range(NIC):
                    nc.tensor.matmul(st_[f"acc{ic}_0"][:],
                                     v[:, j2, bass.ts(hA, D + 1)],
                                     ptA[:, j2, bass.ts(ic, 512)], **stf)
                for ic in range(NIC):
                    nc.tensor.matmul(st_[f"acc{ic}_1"][:],
                                     v[:, j2, bass.ts(hB, D + 1)],
                                     ptB[:, j2, bass.ts(ic, 512)], **stf)

            if jc == 3:
                for ic in range(NIC):
                    for h, nm_ in ((0, "A"), (1, "B")):
                        st_[f"acc{ic}_{h}"] = opool.tile(
                            [65, 512], F32, tag="o", name=f"o{ic}{nm_}{m}")
                for j2 in range(4):
                    o_mms(j2, first=(j2 == 0))
            elif jc > 3:
                o_mms(jc, first=False)

        # ---------------- flush m=7 + out-projection ----------------
        normalize_m(NC - 1)

        with tc.tile_pool(name="woch", bufs=2) as wochp, \
             tc.tile_pool(name="zsb", bufs=1) as zsbp:
            for cc in range(NC):
                wo_sb = wochp.tile([128, NC * 128], F16, tag="wo", name=f"wo{cc}")
                nc.sync.dma_start(wo_sb[:], wblock(wout_t, cc))
                ps = bigp.tile([128, T], F32, tag="big", name=f"z{cc}")
                # m=7 term last: its O^T lands during the flush above
                mm_order = list(range(NC - 1)) + [NC - 1]
                for i, mm in enumerate(mm_order):
                    for ic in range(NIC):
                        nc.tensor.matmul(
                            ps[:, bass.ts(ic, 512)],
                            wo_sb[:, bass.ts(mm, 128)],
                            ot[:, mm, bass.ts(ic, 512)],
                            start=(i == 0), stop=(i == NC - 1))
                zsb = zsbp.tile([128, T], F32, tag="zsb", name=f"zsb{cc}")
                nc.vector.tensor_copy(zsb[:], ps[:])
                nc.sync.dma_start(z_t[cc * 128:(cc + 1) * 128, :], zsb[:])
    nc.compile()
    return nc


def _group_w(wqkv, k):
    """Rows of Wqkv (3E, E) for q/k/v (k=0/1/2), grouped head-major.

    Row index layout: r = di*(3H) + k*H + h  ->  grouped[h*D+di, :].
    """
    w = np.asarray(wqkv, dtype=np.float32).reshape(D, 3, H, E)[:, k]   # [di, h, e]
    return np.ascontiguousarray(w.transpose(1, 0, 2).reshape(E, E))    # [h*D+di, e]


def _stream_layout(w_t):
    """Swizzle w^T [e*128+p, m*128+c] -> [m*128+p, e*128+c] so the device can
    stream output-block m as one contiguous [128, 1024] DMA."""
    a = np.asarray(w_t).reshape(NC, 128, NC, 128)
    return np.ascontiguousarray(a.transpose(2, 1, 0, 3).reshape(E, E))


def kernel(x, y, Wqkv1, Wqkv2, Wout1, Wout2):
    x = np.asarray(x, dtype=np.float32)
    y = np.asarray(y, dtype=np.float32)

    if "nc" not in _NC_CACHE:
        _NC_CACHE["nc"] = _build()
    nc = _NC_CACHE["nc"]

    # weight prep (host): grouped + transposed (f16 on-device dtype)
    wq1_t = np.ascontiguousarray(_group_w(Wqkv1, 0).T)
    wk1_t = np.ascontiguousarray(_group_w(Wqkv1, 1).T)
    wv1_t = np.ascontiguousarray(_group_w(Wqkv1, 2).T)
    wq2_t = np.ascontiguousarray(_group_w(Wqkv2, 0).T)
    wk2_t = np.ascontiguousarray(_group_w(Wqkv2, 1).T)
    wv2_t = np.ascontiguousarray(_group_w(Wqkv2, 2).T)
    wout1_t = np.ascontiguousarray(np.asarray(Wout1, dtype=np.float32).T)
    wout2_t = np.ascontiguousarray(np.asarray(Wout2, dtype=np.float32).T)

    in_maps = []
    for c in range(N_CORES):
        s, b = divmod(c, B)
        if s == 0:
            # stream-1 output: K,V from x via Wqkv1; Q from y via Wqkv2
            a_t, b_t = x[b].T, y[b].T
            wq, wk, wv, wo = wq2_t, wk1_t, wv1_t, wout1_t
        else:
            a_t, b_t = y[b].T, x[b].T
            wq, wk, wv, wo = wq1_t, wk2_t, wv2_t, wout2_t
        in_maps.append({
            "a_t": np.ascontiguousarray(a_t).astype(np.float16),
            "b_t": np.ascontiguousarray(b_t).astype(np.float16),
            "wq_t": _stream_layout(wq).astype(np.float16),
            "wk_t": _stream_layout(wk).astype(np.float16),
            "wv_t": wv.astype(np.float16),
            "wout_t": _stream_layout(wo).astype(np.float16),
        })

    trace = os.environ.get("BASS_KERNEL_TRACE", "0") == "1"
    if trace:
        try:
            from antenv.axon_hooks import get_axon_ntff_profile_hook  # noqa: F401
        except ImportError:
            trace = False
    ncores = int(os.environ.get("KCORES", str(N_CORES)))
    r = bass_utils.run_bass_kernel_spmd(nc, in_maps[:ncores], core_ids=list(range(ncores)),
                                        trace=trace)
    LAST_RESULTS["exec_time_ns"] = r.exec_time_ns
    LAST_RESULTS["profile_json"] = r.profile_json

    out1 = np.stack([r.results[b]["z_t"].T for b in range(B)]).astype(np.float32)
    out2 = np.stack([r.results[B + b]["z_t"].T for b in range(B)]).astype(np.float32)
    return out1, out2


# revision 30
# speedup vs baseline: 1.4768x; 1.1016x over previous
"""Trainium2 Bass kernel for nn_MultiHeadCrossAttention (B=4, T=1024, E=1024, H=16).

Sharding: 8 fully independent shards (output stream s, batch b), zero
cross-core communication. Stream-1 output xo@Wout1 needs K,V from x and Q
from y; stream-2 the reverse. Core c<4 computes stream-1 batch c; core c>=4
stream-2 batch c-4.

Per-core design: one flat software-pipelined stream over 64 (head-pair m,
key-chunk jc) units, paced by ScalarE exp (the irreducible ~147us of
softmax exponentials). Everything else hides inside that window so the
tensor engine never idles >3.4us (HAM stays at full clock):
  unit (m, jc): S^T = K^T.T @ Q^T (row-paired K=64 MMs, [128,T] PSUM);
                P^T = exp(S^T/8) (2 ACTs); O' ic0-half accumulates
                ([65,512] PSUM, ones-row gives rowsums for free).
  injections (relative to unit stream):
    m=0 units carry V-projection for chunk jc just-in-time.
    (m,0)/(m,1): O' ic1-half of m-1 (A then B, staggered for PSUM slots);
    (m,1)/(m,2): normalize m-1 (reciprocal_approx_fast on free-dim rowsums,
                 K=1 ones-matmul broadcast, DVE mul into O^T f16);
    (m,4)/(m,6): Q^T/K^T projections for chunk m+1 (weights streamed from
                 HBM in host-swizzled contiguous blocks).
  tail: flush m=7, then Z^T = Wout^T.T @ O^T per 128-row chunk (m=7 term
  accumulated last so Z overlaps the flush).
PSUM budget (8 banks): S units 2x[128,1024]=4, O' accums 3x[65,512]=3,
proj/bc scratch 1x[128,512]=1.
Host pre-transposes/groups weights and activations; re-transposes outputs.
"""

import os
import sys

sys.path.insert(0, "/opt/trn_rl_repo")

import numpy as np
import ml_dtypes
from contextlib import ExitStack

import concourse.bass as bass
import concourse.mybir as mybir
import concourse.tile as tile
from concourse import bacc
from concourse import bass_utils

B, T, E, H = 4, 1024, 1024, 16
D = E // H            # 64
NC = E // 128         # 8 chunks of 128
NIC = T // 512        # 2 free-dim chunks of 512
N_CORES = 8

F32 = mybir.dt.float32
F16 = mybir.dt.float16
EXP = mybir.ActivationFunctionType.Exp

_NC_CACHE = {}
LAST_RESULTS = {}


def _build():
    nc = bacc.Bacc("TRN2", target_bir_lowering=False, debug=False,
                   enable_asserts=False, num_devices=N_CORES)
    a_t = nc.dram_tensor("a_t", (E, T), F16, kind="ExternalInput").ap()
    b_t = nc.dram_tensor("b_t", (E, T), F16, kind="ExternalInput").ap()
    wq_t = nc.dram_tensor("wq_t", (E, E), F16, kind="ExternalInput").ap()
    wk_t = nc.dram_tensor("wk_t", (E, E), F16, kind="ExternalInput").ap()
    wv_t = nc.dram_tensor("wv_t", (E, E), F16, kind="ExternalInput").ap()
    wout_t = nc.dram_tensor("wout_t", (E, E), F16, kind="ExternalInput").ap()
    z_t = nc.dram_tensor("z_t", (E, T), F32, kind="ExternalOutput").ap()

    # wq_t/wk_t/wout_t are host-swizzled: block m of w^T is the contiguous
    # [128, 1024] slice rows m*128..(m+1)*128, laid out [p, e*128+c].
    def wblock(w, m):
        return w[m * 128:(m + 1) * 128, :]

    with tile.TileContext(nc) as tc, ExitStack() as ctx:
        persist = ctx.enter_context(tc.tile_pool(name="persist", bufs=1))
        qt = persist.tile([128, NC, T], F16, tag="qt")
        kt = persist.tile([128, NC, T], F16, tag="kt")
        v = persist.tile([128, NC, H * (D + 1)], F16, tag="v")
        at_sb = persist.tile([128, NC, T], F16, tag="at")
        bt_sb = persist.tile([128, NC, T], F16, tag="bt")
        ot = persist.tile([128, NC, T], F16, tag="ot")      # normalized O^T
        ones_t = persist.tile([128, 128], F16, tag="ones")
        nc.vector.memset(ones_t[:], 1.0)

        # DMA order matters: bt + first weight chunks first so Q0/K0 start
        # early; at/wv interleaved so V-proj can chase the transfers.
        wch = ctx.enter_context(tc.tile_pool(name="wch", bufs=2))
        wq0 = wch.tile([128, NC * 128], F16, tag="w", name="wq0")
        wk0 = wch.tile([128, NC * 128], F16, tag="w", name="wk0")
        nc.sync.dma_start(wq0[:], wblock(wq_t, 0))
        nc.sync.dma_start(wk0[:], wblock(wk_t, 0))
        for c in range(NC):
            nc.sync.dma_start(bt_sb[:, c, :], b_t[c * 128:(c + 1) * 128, :])
            nc.sync.dma_start(at_sb[:, c, :], a_t[c * 128:(c + 1) * 128, :])

        wvp = ctx.enter_context(tc.tile_pool(name="wv", bufs=1))
        wv_sb = wvp.tile([128, NC, E], F16, tag="wv")
        for c in range(NC):
            nc.sync.dma_start(wv_sb[:, c, :], wv_t[c * 128:(c + 1) * 128, :])

        # ones column per head in V (col D within each D+1 group) -> rowsums
        for m in range(NC):
            nc.vector.memset(
                v[:, m, :].rearrange("p (h x) -> p h x", x=D + 1)[:, :, D:D + 1], 1.0)

        ptp = ctx.enter_context(tc.tile_pool(name="pt", bufs=3))
        rsp = ctx.enter_context(tc.tile_pool(name="rsp", bufs=1))
        bcp = ctx.enter_context(tc.tile_pool(name="bcp", bufs=1))
        bigp = ctx.enter_context(tc.tile_pool(name="bigp", bufs=2, space="PSUM"))
        opool = ctx.enter_context(tc.tile_pool(name="op", bufs=4, space="PSUM"))

        def proj_chunk(w_sb, act_sb, out_sb, m, on_act, use_opool=False):
            """One [128, T] projection chunk (contract over e, weight block
            loaded once, both ic-half accumulators live)."""
            if use_opool:
                ps0 = opool.tile([128, 512], F32, tag="o", name=f"pj{m}a")
                ps1 = opool.tile([128, 512], F32, tag="o", name=f"pj{m}b")
            else:
                big = bigp.tile([128, T], F32, tag="big", name=f"pj{m}")
                ps0, ps1 = big[:, 0:512], big[:, 512:1024]
            for e in range(NC):
                st_e = dict(start=(e == 0), stop=(e == NC - 1))
                nc.tensor.matmul(ps0[:, :] if use_opool else ps0,
                                 w_sb[:, bass.ts(e, 128)],
                                 act_sb[:, e, 0:512], **st_e)
                nc.tensor.matmul(ps1[:, :] if use_opool else ps1,
                                 w_sb[:, bass.ts(e, 128)],
                                 act_sb[:, e, 512:1024], **st_e)
            for ic, ps in ((0, ps0), (1, ps1)):
                dst = out_sb[:, m, bass.ts(ic, 512)]
                if on_act:
                    nc.scalar.copy(dst, ps[:, :] if use_opool else ps)
                else:
                    nc.vector.tensor_copy(dst, ps[:, :] if use_opool else ps)

        def vproj_chunk(jc):
            """V chunk jc (natural layout), copy strided into (h, D+1)."""
            ps = bigp.tile([128, T], F32, tag="big", name=f"vps{jc}")
            for e in range(NC):
                for ic in range(NIC):
                    nc.tensor.matmul(
                        ps[:, bass.ts(ic, 512)], at_sb[:, e, bass.ts(jc, 128)],
                        wv_sb[:, e, bass.ts(ic, 512)],
                        start=(e == 0), stop=(e == NC - 1))
            nc.vector.tensor_copy(
                v[:, jc, :].rearrange("p (h x) -> p h x", x=D + 1)[:, :, 0:D],
                ps[:].rearrange("p (h x) -> p h x", x=D))

        proj_chunk(wq0, bt_sb, qt, 0, on_act=True)
        proj_chunk(wk0, at_sb, kt, 0, on_act=True)

        # per-m pipeline state
        state = {}

        def normalize_m(m):
            """rowsums -> 1/r -> broadcast -> O^T = O' * (1/r), both ic halves.

            Frees all four of m's O'-accumulator PSUM tiles. The four rowsum
            rows are spread to partitions 0/32/64/96 so the reciprocal runs
            512 elements wide instead of 2048 serial. Its bc matmuls are
            emitted before any PE instruction that waits on those slots.
            """
            st_ = state[m]
            rsf = rsp.tile([128, 512], F32, tag="rsf", name=f"rsf{m}")
            rsr = rsp.tile([128, 512], F32, tag="rsr", name=f"rsr{m}")
            rsr16 = rsp.tile([128, 512], F16, tag="rsr16", name=f"rsr16_{m}")
            rows = {(0, 0): 0, (0, 1): 32, (1, 0): 64, (1, 1): 96}
            for (h, ic), p in rows.items():
                nc.vector.tensor_copy(rsf[p:p + 1, :],
                                      st_[f"acc{ic}_{h}"][64:65, :])
            nc.vector.reciprocal_approx_fast(rsr[:], rsf[:])
            with nc.allow_low_precision(reason="1/r feeds f16 bc matmul"):
                nc.vector.tensor_copy(rsr16[:], rsr[:])
            bc = bigp.tile([128, T], F32, tag="big", name=f"bc{m}")
            for (h, ic), p in rows.items():
                nc.tensor.matmul(
                    bc[h * 64:(h + 1) * 64, bass.ts(ic, 512)],
                    ones_t[p:p + 1, 0:64], rsr16[p:p + 1, :],
                    start=True, stop=True, tile_position=(p, h * 64))
            bcs = bcp.tile([128, T], F32, tag="bcs", name=f"bcs{m}")
            nc.vector.tensor_copy(bcs[:], bc[:])
            with nc.allow_low_precision(reason="O^T f16 feeds f16 out-proj"):
                for ic in range(NIC):
                    s_ic = bass.ts(ic, 512)
                    nc.vector.tensor_mul(ot[0:64, m, s_ic],
                                         st_[f"acc{ic}_0"][0:64, :], bcs[0:64, s_ic])
                    nc.vector.tensor_mul(ot[64:128, m, s_ic],
                                         st_[f"acc{ic}_1"][0:64, :],
                                         bcs[64:128, s_ic])

        # ---------------- the 64-unit stream ----------------
        for u in range(NC * NC):
            m, jc = divmod(u, NC)
            hA, hB = 2 * m, 2 * m + 1
            if jc == 0:
                state[m] = {
                    "ptA": ptp.tile([128, NC, T], F16, tag="pt", name=f"ptA{m}"),
                    "ptB": ptp.tile([128, NC, T], F16, tag="pt", name=f"ptB{m}"),
                }
            st_ = state[m]
            ptA, ptB = st_["ptA"], st_["ptB"]

            ps_s = bigp.tile([128, T], F32, tag="big", name=f"sA{u}")
            ps_sB = bigp.tile([128, T], F32, tag="big", name=f"sB{u}")
            for ic in range(NIC):
                nc.tensor.matmul(
                    ps_s[:, bass.ts(ic, 512)],
                    kt[0:64, m, bass.ts(jc, 128)],
                    qt[0:64, m, bass.ts(ic, 512)],
                    start=True, stop=True)
            for ic in range(NIC):
                nc.tensor.matmul(
                    ps_sB[:, bass.ts(ic, 512)],
                    kt[64:128, m, bass.ts(jc, 128)],
                    qt[64:128, m, bass.ts(ic, 512)],
                    start=True, stop=True, tile_position=(64, 0))
            nc.scalar.activation(ptA[:, jc, :], ps_s[:], EXP, scale=0.125)
            nc.scalar.activation(ptB[:, jc, :], ps_sB[:], EXP, scale=0.125)

            if m == 0:
                vproj_chunk(jc)   # just-in-time V for O' below

            # ---- injections (pipelined work of m-1 / m+1); emitted BEFORE
            # this unit's O' matmuls so normalize's bc matmuls precede any
            # PE instruction that waits on the slots its muls release ----
            if jc == 0 and m >= 1:
                normalize_m(m - 1)
                del state[m - 1]
            elif jc == 1 and m >= 1 and m + 1 < NC:
                wq_sb = wch.tile([128, NC * 128], F16, tag="w", name=f"wq{m+1}")
                nc.sync.dma_start(wq_sb[:], wblock(wq_t, m + 1))
                proj_chunk(wq_sb, bt_sb, qt, m + 1, on_act=False, use_opool=True)
            elif jc == 2 and m >= 1 and m + 1 < NC:
                wk_sb = wch.tile([128, NC * 128], F16, tag="w", name=f"wk{m+1}")
                nc.sync.dma_start(wk_sb[:], wblock(wk_t, m + 1))
                proj_chunk(wk_sb, at_sb, kt, m + 1, on_act=False, use_opool=True)
            elif jc == 4 and m == 0:
                wq_sb = wch.tile([128, NC * 128], F16, tag="w", name="wq1")
                nc.sync.dma_start(wq_sb[:], wblock(wq_t, 1))
                proj_chunk(wq_sb, bt_sb, qt, 1, on_act=False)
            elif jc == 6 and m == 0:
                wk_sb = wch.tile([128, NC * 128], F16, tag="w", name="wk1")
                nc.sync.dma_start(wk_sb[:], wblock(wk_t, 1))
                proj_chunk(wk_sb, at_sb, kt, 1, on_act=False)

            # O' accumulation (both ic halves per weight load). Start
            # deferred to jc==4 so the slot wait lands after proj(m+1) has
            # released the time-shared accumulator slots.
            def o_mms(j2, first):
                stf = dict(start=first, stop=(j2 == NC - 1))
                for ic in range(NIC):
                    nc.tensor.matmul(st_[f"acc{ic}_0"][0:65, :],
                                     v[:, j2, bass.ts(hA, D + 1)],
                                     ptA[:, j2, bass.ts(ic, 512)], **stf)
                for ic in range(NIC):
                    nc.tensor.matmul(st_[f"acc{ic}_1"][0:65, :],
                                     v[:, j2, bass.ts(hB, D + 1)],
                                     ptB[:, j2, bass.ts(ic, 512)], **stf)

            if jc == 4:
                for ic in range(NIC):
                    for h, nm_ in ((0, "A"), (1, "B")):
                        st_[f"acc{ic}_{h}"] = opool.tile(
                            [128, 512], F32, tag="o", name=f"o{ic}{nm_}{m}")
                for j2 in range(5):
                    o_mms(j2, first=(j2 == 0))
            elif jc > 4:
                o_mms(jc, first=False)

        # ---------------- flush m=7 + out-projection ----------------
        normalize_m(NC - 1)

        with tc.tile_pool(name="woch", bufs=2) as wochp, \
             tc.tile_pool(name="zsb", bufs=1) as zsbp:
            for cc in range(NC):
                wo_sb = wochp.tile([128, NC * 128], F16, tag="wo", name=f"wo{cc}")
                nc.sync.dma_start(wo_sb[:], wblock(wout_t, cc))
                ps = bigp.tile([128, T], F32, tag="big", name=f"z{cc}")
                # m=7 term last: its O^T lands during the flush above
                for i, mm in enumerate(range(NC)):
                    for ic in range(NIC):
                        nc.tensor.matmul(
                            ps[:, bass.ts(ic, 512)],
                            wo_sb[:, bass.ts(mm, 128)],
                            ot[:, mm, bass.ts(ic, 512)],
                            start=(i == 0), stop=(i == NC - 1))
                zsb = zsbp.tile([128, T], F32, tag="zsb", name=f"zsb{cc}")
                nc.vector.tensor_copy(zsb[:], ps[:])
                nc.sync.dma_start(z_t[cc * 128:(cc + 1) * 128, :], zsb[:])
    nc.compile()
    return nc


def _group_w(wqkv, k):
    """Rows of Wqkv (3E, E) for q/k/v (k=0/1/2), grouped head-major.

    Row index layout: r = di*(3H) + k*H + h  ->  grouped[h*D+di, :].
    """
    w = np.asarray(wqkv, dtype=np.float32).reshape(D, 3, H, E)[:, k]   # [di, h, e]
    return np.ascontiguousarray(w.transpose(1, 0, 2).reshape(E, E))    # [h*D+di, e]


def _stream_layout(w_t):
    """Swizzle w^T [e*128+p, m*128+c] -> [m*128+p, e*128+c] so the device can
    stream output-block m as one contiguous [128, 1024] DMA."""
    a = np.asarray(w_t).reshape(NC, 128, NC, 128)
    return np.ascontiguousarray(a.transpose(2, 1, 0, 3).reshape(E, E))


def kernel(x, y, Wqkv1, Wqkv2, Wout1, Wout2):
    x = np.asarray(x, dtype=np.float32)
    y = np.asarray(y, dtype=np.float32)

    if "nc" not in _NC_CACHE:
        _NC_CACHE["nc"] = _build()
    nc = _NC_CACHE["nc"]

    # weight prep (host): grouped + transposed (f16 on-device dtype)
    wq1_t = np.ascontiguousarray(_group_w(Wqkv1, 0).T)
    wk1_t = np.ascontiguousarray(_group_w(Wqkv1, 1).T)
    wv1_t = np.ascontiguousarray(_group_w(Wqkv1, 2).T)
    wq2_t = np.ascontiguousarray(_group_w(Wqkv2, 0).T)
    wk2_t = np.ascontiguousarray(_group_w(Wqkv2, 1).T)
    wv2_t = np.ascontiguousarray(_group_w(Wqkv2, 2).T)
    wout1_t = np.ascontiguousarray(np.asarray(Wout1, dtype=np.float32).T)
    wout2_t = np.ascontiguousarray(np.asarray(Wout2, dtype=np.float32).T)

    in_maps = []
    for c in range(N_CORES):
        s, b = divmod(c, B)
        if s == 0:
            # stream-1 output: K,V from x via Wqkv1; Q from y via Wqkv2
            a_t, b_t = x[b].T, y[b].T
            wq, wk, wv, wo = wq2_t, wk1_t, wv1_t, wout1_t
        else:
            a_t, b_t = y[b].T, x[b].T
            wq, wk, wv, wo = wq1_t, wk2_t, wv2_t, wout2_t
        in_maps.append({
            "a_t": np.ascontiguousarray(a_t).astype(np.float16),
            "b_t": np.ascontiguousarray(b_t).astype(np.float16),
            "wq_t": _stream_layout(wq).astype(np.float16),
            "wk_t": _stream_layout(wk).astype(np.float16),
            "wv_t": wv.astype(np.float16),
            "wout_t": _stream_layout(wo).astype(np.float16),
        })

    trace = os.environ.get("BASS_KERNEL_TRACE", "0") == "1"
    if trace:
        try:
            from antenv.axon_hooks import get_axon_ntff_profile_hook  # noqa: F401
        except ImportError:
            trace = False
    ncores = int(os.environ.get("KCORES", str(N_CORES)))
    r = bass_utils.run_bass_kernel_spmd(nc, in_maps[:ncores], core_ids=list(range(ncores)),
                                        trace=trace)
    LAST_RESULTS["exec_time_ns"] = r.exec_time_ns
    LAST_RESULTS["profile_json"] = r.profile_json

    out1 = np.stack([r.results[b]["z_t"].T for b in range(B)]).astype(np.float32)
    out2 = np.stack([r.results[B + b]["z_t"].T for b in range(B)]).astype(np.float32)
    return out1, out2


# revision 31
# speedup vs baseline: 1.5183x; 1.0281x over previous
"""Trainium2 Bass kernel for nn_MultiHeadCrossAttention (B=4, T=1024, E=1024, H=16).

Sharding: 8 fully independent shards (output stream s, batch b), zero
cross-core communication. Stream-1 output xo@Wout1 needs K,V from x and Q
from y; stream-2 the reverse. Core c<4 computes stream-1 batch c; core c>=4
stream-2 batch c-4.

Per-core design: one flat software-pipelined stream over 64 (head-pair m,
key-chunk jc) units, paced by ScalarE exp (the irreducible ~147us of
softmax exponentials). Everything else hides inside that window so the
tensor engine never idles >3.4us (HAM stays at full clock):
  unit (m, jc): S^T = K^T.T @ Q^T (row-paired K=64 MMs, [128,T] PSUM);
                P^T = exp(S^T/8) (2 ACTs); O' ic0-half accumulates
                ([65,512] PSUM, ones-row gives rowsums for free).
  injections (relative to unit stream):
    m=0 units carry V-projection for chunk jc just-in-time.
    (m,0)/(m,1): O' ic1-half of m-1 (A then B, staggered for PSUM slots);
    (m,1)/(m,2): normalize m-1 (reciprocal_approx_fast on free-dim rowsums,
                 K=1 ones-matmul broadcast, DVE mul into O^T f16);
    (m,4)/(m,6): Q^T/K^T projections for chunk m+1 (weights streamed from
                 HBM in host-swizzled contiguous blocks).
  tail: flush m=7, then Z^T = Wout^T.T @ O^T per 128-row chunk (m=7 term
  accumulated last so Z overlaps the flush).
PSUM budget (8 banks): S units 2x[128,1024]=4, O' accums 3x[65,512]=3,
proj/bc scratch 1x[128,512]=1.
Host pre-transposes/groups weights and activations; re-transposes outputs.
"""

import os
import sys

sys.path.insert(0, "/opt/trn_rl_repo")

import numpy as np
import ml_dtypes
from contextlib import ExitStack

import concourse.bass as bass
import concourse.mybir as mybir
import concourse.tile as tile
from concourse import bacc
from concourse import bass_utils

B, T, E, H = 4, 1024, 1024, 16
D = E // H            # 64
NC = E // 128         # 8 chunks of 128
NIC = T // 512        # 2 free-dim chunks of 512
N_CORES = 8

F32 = mybir.dt.float32
F16 = mybir.dt.float16
EXP = mybir.ActivationFunctionType.Exp

_NC_CACHE = {}
LAST_RESULTS = {}


def _build():
    nc = bacc.Bacc("TRN2", target_bir_lowering=False, debug=False,
                   enable_asserts=False, num_devices=N_CORES)
    a_t = nc.dram_tensor("a_t", (E, T), F16, kind="ExternalInput").ap()
    b_t = nc.dram_tensor("b_t", (E, T), F16, kind="ExternalInput").ap()
    wq_t = nc.dram_tensor("wq_t", (E, E), F16, kind="ExternalInput").ap()
    wk_t = nc.dram_tensor("wk_t", (E, E), F16, kind="ExternalInput").ap()
    wv_t = nc.dram_tensor("wv_t", (E, E), F16, kind="ExternalInput").ap()
    wout_t = nc.dram_tensor("wout_t", (E, E), F16, kind="ExternalInput").ap()
    z_t = nc.dram_tensor("z_t", (E, T), F32, kind="ExternalOutput").ap()

    # wq_t/wk_t/wout_t are host-swizzled: block m of w^T is the contiguous
    # [128, 1024] slice rows m*128..(m+1)*128, laid out [p, e*128+c].
    def wblock(w, m):
        return w[m * 128:(m + 1) * 128, :]

    with tile.TileContext(nc) as tc, ExitStack() as ctx:
        persist = ctx.enter_context(tc.tile_pool(name="persist", bufs=1))
        qt = persist.tile([128, NC, T], F16, tag="qt")
        kt = persist.tile([128, NC, T], F16, tag="kt")
        v = persist.tile([128, NC, H * (D + 1)], F16, tag="v")
        at_sb = persist.tile([128, NC, T], F16, tag="at")
        bt_sb = persist.tile([128, NC, T], F16, tag="bt")
        ot = persist.tile([128, NC, T], F16, tag="ot")      # normalized O^T
        ones_t = persist.tile([128, 128], F16, tag="ones")
        nc.vector.memset(ones_t[:], 1.0)

        # DMA order matters: bt + first weight chunks first so Q0/K0 start
        # early; at/wv interleaved so V-proj can chase the transfers.
        wch = ctx.enter_context(tc.tile_pool(name="wch", bufs=2))
        wq0 = wch.tile([128, NC * 128], F16, tag="w", name="wq0")
        wk0 = wch.tile([128, NC * 128], F16, tag="w", name="wk0")
        for c in range(NC):
            nc.sync.dma_start(bt_sb[:, c, :], b_t[c * 128:(c + 1) * 128, :])
        nc.sync.dma_start(wq0[:], wblock(wq_t, 0))
        nc.sync.dma_start(wk0[:], wblock(wk_t, 0))
        for c in range(NC):
            nc.sync.dma_start(at_sb[:, c, :], a_t[c * 128:(c + 1) * 128, :])

        wvp = ctx.enter_context(tc.tile_pool(name="wv", bufs=1))
        wv_sb = wvp.tile([128, NC, E], F16, tag="wv")
        for c in range(NC):
            nc.sync.dma_start(wv_sb[:, c, :], wv_t[c * 128:(c + 1) * 128, :])

        # ones column per head in V (col D within each D+1 group) -> rowsums
        for m in range(NC):
            nc.vector.memset(
                v[:, m, :].rearrange("p (h x) -> p h x", x=D + 1)[:, :, D:D + 1], 1.0)

        ptp = ctx.enter_context(tc.tile_pool(name="pt", bufs=4))
        rsp = ctx.enter_context(tc.tile_pool(name="rsp", bufs=1))
        bcp = ctx.enter_context(tc.tile_pool(name="bcp", bufs=1))
        bigp = ctx.enter_context(tc.tile_pool(name="bigp", bufs=2, space="PSUM"))
        opool = ctx.enter_context(tc.tile_pool(name="op", bufs=4, space="PSUM"))

        def proj_chunk(w_sb, act_sb, out_sb, m, on_act, use_opool=False):
            """One [128, T] projection chunk (contract over e, weight block
            loaded once, both ic-half accumulators live)."""
            if use_opool:
                ps0 = opool.tile([128, 512], F32, tag="o", name=f"pj{m}a")
                ps1 = opool.tile([128, 512], F32, tag="o", name=f"pj{m}b")
            else:
                big = bigp.tile([128, T], F32, tag="big", name=f"pj{m}")
                ps0, ps1 = big[:, 0:512], big[:, 512:1024]
            for e in range(NC):
                st_e = dict(start=(e == 0), stop=(e == NC - 1))
                nc.tensor.matmul(ps0[:, :] if use_opool else ps0,
                                 w_sb[:, bass.ts(e, 128)],
                                 act_sb[:, e, 0:512], **st_e)
                nc.tensor.matmul(ps1[:, :] if use_opool else ps1,
                                 w_sb[:, bass.ts(e, 128)],
                                 act_sb[:, e, 512:1024], **st_e)
            for ic, ps in ((0, ps0), (1, ps1)):
                dst = out_sb[:, m, bass.ts(ic, 512)]
                if on_act:
                    nc.scalar.copy(dst, ps[:, :] if use_opool else ps)
                else:
                    nc.vector.tensor_copy(dst, ps[:, :] if use_opool else ps)

        def vproj_chunk(jc):
            """V chunk jc (natural layout), copy strided into (h, D+1)."""
            ps = bigp.tile([128, T], F32, tag="big", name=f"vps{jc}")
            for e in range(NC):
                for ic in range(NIC):
                    nc.tensor.matmul(
                        ps[:, bass.ts(ic, 512)], at_sb[:, e, bass.ts(jc, 128)],
                        wv_sb[:, e, bass.ts(ic, 512)],
                        start=(e == 0), stop=(e == NC - 1))
            nc.vector.tensor_copy(
                v[:, jc, :].rearrange("p (h x) -> p h x", x=D + 1)[:, :, 0:D],
                ps[:].rearrange("p (h x) -> p h x", x=D))

        proj_chunk(wq0, bt_sb, qt, 0, on_act=True)
        proj_chunk(wk0, at_sb, kt, 0, on_act=True)

        # per-m pipeline state
        state = {}

        def normalize_m(m):
            """rowsums -> 1/r -> broadcast -> O^T = O' * (1/r), both ic halves.

            Frees all four of m's O'-accumulator PSUM tiles. The four rowsum
            rows are spread to partitions 0/32/64/96 so the reciprocal runs
            512 elements wide instead of 2048 serial. Its bc matmuls are
            emitted before any PE instruction that waits on those slots.
            """
            st_ = state[m]
            rsf = rsp.tile([128, 512], F32, tag="rsf", name=f"rsf{m}")
            rsr = rsp.tile([128, 512], F32, tag="rsr", name=f"rsr{m}")
            rsr16 = rsp.tile([128, 512], F16, tag="rsr16", name=f"rsr16_{m}")
            rows = {(0, 0): 0, (0, 1): 32, (1, 0): 64, (1, 1): 96}
            for (h, ic), p in rows.items():
                nc.vector.tensor_copy(rsf[p:p + 1, :],
                                      st_[f"acc{ic}_{h}"][64:65, :])
            nc.vector.reciprocal_approx_fast(rsr[:], rsf[:])
            with nc.allow_low_precision(reason="1/r feeds f16 bc matmul"):
                nc.vector.tensor_copy(rsr16[:], rsr[:])
            bc = bigp.tile([128, T], F32, tag="big", name=f"bc{m}")
            for (h, ic), p in rows.items():
                nc.tensor.matmul(
                    bc[h * 64:(h + 1) * 64, bass.ts(ic, 512)],
                    ones_t[p:p + 1, 0:64], rsr16[p:p + 1, :],
                    start=True, stop=True, tile_position=(p, h * 64))
            bcs = bcp.tile([128, T], F32, tag="bcs", name=f"bcs{m}")
            nc.vector.tensor_copy(bcs[:], bc[:])
            with nc.allow_low_precision(reason="O^T f16 feeds f16 out-proj"):
                for ic in range(NIC):
                    s_ic = bass.ts(ic, 512)
                    nc.vector.tensor_mul(ot[0:64, m, s_ic],
                                         st_[f"acc{ic}_0"][0:64, :], bcs[0:64, s_ic])
                    nc.vector.tensor_mul(ot[64:128, m, s_ic],
                                         st_[f"acc{ic}_1"][0:64, :],
                                         bcs[64:128, s_ic])

        # ---------------- the 64-unit stream ----------------
        for u in range(NC * NC):
            m, jc = divmod(u, NC)
            hA, hB = 2 * m, 2 * m + 1
            if jc == 0:
                state[m] = {
                    "ptA": ptp.tile([128, NC, T], F16, tag="pt", name=f"ptA{m}"),
                    "ptB": ptp.tile([128, NC, T], F16, tag="pt", name=f"ptB{m}"),
                }
            st_ = state[m]
            ptA, ptB = st_["ptA"], st_["ptB"]

            ps_s = bigp.tile([128, T], F32, tag="big", name=f"sA{u}")
            ps_sB = bigp.tile([128, T], F32, tag="big", name=f"sB{u}")
            for ic in range(NIC):
                nc.tensor.matmul(
                    ps_s[:, bass.ts(ic, 512)],
                    kt[0:64, m, bass.ts(jc, 128)],
                    qt[0:64, m, bass.ts(ic, 512)],
                    start=True, stop=True)
            for ic in range(NIC):
                nc.tensor.matmul(
                    ps_sB[:, bass.ts(ic, 512)],
                    kt[64:128, m, bass.ts(jc, 128)],
                    qt[64:128, m, bass.ts(ic, 512)],
                    start=True, stop=True, tile_position=(64, 0))
            nc.scalar.activation(ptA[:, jc, :], ps_s[:], EXP, scale=0.125)
            nc.scalar.activation(ptB[:, jc, :], ps_sB[:], EXP, scale=0.125)

            if m == 0:
                vproj_chunk(jc)   # just-in-time V for O' below

            # ---- injections (pipelined work of m-1 / m+1); emitted BEFORE
            # this unit's O' matmuls so normalize's bc matmuls precede any
            # PE instruction that waits on the slots its muls release ----
            if jc == 0 and m >= 1:
                normalize_m(m - 1)
                del state[m - 1]
            elif jc == 1 and m >= 1 and m + 1 < NC:
                wq_sb = wch.tile([128, NC * 128], F16, tag="w", name=f"wq{m+1}")
                nc.sync.dma_start(wq_sb[:], wblock(wq_t, m + 1))
                proj_chunk(wq_sb, bt_sb, qt, m + 1, on_act=False, use_opool=True)
            elif jc == 2 and m >= 1 and m + 1 < NC:
                wk_sb = wch.tile([128, NC * 128], F16, tag="w", name=f"wk{m+1}")
                nc.sync.dma_start(wk_sb[:], wblock(wk_t, m + 1))
                proj_chunk(wk_sb, at_sb, kt, m + 1, on_act=False, use_opool=True)
            elif jc == 4 and m == 0:
                wq_sb = wch.tile([128, NC * 128], F16, tag="w", name="wq1")
                nc.sync.dma_start(wq_sb[:], wblock(wq_t, 1))
                proj_chunk(wq_sb, bt_sb, qt, 1, on_act=False)
            elif jc == 6 and m == 0:
                wk_sb = wch.tile([128, NC * 128], F16, tag="w", name="wk1")
                nc.sync.dma_start(wk_sb[:], wblock(wk_t, 1))
                proj_chunk(wk_sb, at_sb, kt, 1, on_act=False)

            # O' accumulation (both ic halves per weight load). Start
            # deferred to jc==4 so the slot wait lands after proj(m+1) has
            # released the time-shared accumulator slots.
            def o_mms(j2, first):
                stf = dict(start=first, stop=(j2 == NC - 1))
                for ic in range(NIC):
                    nc.tensor.matmul(st_[f"acc{ic}_0"][0:65, :],
                                     v[:, j2, bass.ts(hA, D + 1)],
                                     ptA[:, j2, bass.ts(ic, 512)], **stf)
                for ic in range(NIC):
                    nc.tensor.matmul(st_[f"acc{ic}_1"][0:65, :],
                                     v[:, j2, bass.ts(hB, D + 1)],
                                     ptB[:, j2, bass.ts(ic, 512)], **stf)

            if jc == 4:
                for ic in range(NIC):
                    for h, nm_ in ((0, "A"), (1, "B")):
                        st_[f"acc{ic}_{h}"] = opool.tile(
                            [128, 512], F32, tag="o", name=f"o{ic}{nm_}{m}")
                for j2 in range(5):
                    o_mms(j2, first=(j2 == 0))
            elif jc > 4:
                o_mms(jc, first=False)

        # ---------------- flush m=7 + out-projection ----------------
        normalize_m(NC - 1)

        with tc.tile_pool(name="woch", bufs=3) as wochp, \
             tc.tile_pool(name="zsb", bufs=2) as zsbp:
            for cc in range(NC):
                wo_sb = wochp.tile([128, NC * 128], F16, tag="wo", name=f"wo{cc}")
                nc.sync.dma_start(wo_sb[:], wblock(wout_t, cc))
                ps = bigp.tile([128, T], F32, tag="big", name=f"z{cc}")
                # m=7 term last: its O^T lands during the flush above
                for i, mm in enumerate(range(NC)):
                    for ic in range(NIC):
                        nc.tensor.matmul(
                            ps[:, bass.ts(ic, 512)],
                            wo_sb[:, bass.ts(mm, 128)],
                            ot[:, mm, bass.ts(ic, 512)],
                            start=(i == 0), stop=(i == NC - 1))
                zsb = zsbp.tile([128, T], F32, tag="zsb", name=f"zsb{cc}")
                nc.vector.tensor_copy(zsb[:], ps[:])
                nc.sync.dma_start(z_t[cc * 128:(cc + 1) * 128, :], zsb[:])
    nc.compile()
    return nc


def _group_w(wqkv, k):
    """Rows of Wqkv (3E, E) for q/k/v (k=0/1/2), grouped head-major.

    Row index layout: r = di*(3H) + k*H + h  ->  grouped[h*D+di, :].
    """
    w = np.asarray(wqkv, dtype=np.float32).reshape(D, 3, H, E)[:, k]   # [di, h, e]
    return np.ascontiguousarray(w.transpose(1, 0, 2).reshape(E, E))    # [h*D+di, e]


def _stream_layout(w_t):
    """Swizzle w^T [e*128+p, m*128+c] -> [m*128+p, e*128+c] so the device can
    stream output-block m as one contiguous [128, 1024] DMA."""
    a = np.asarray(w_t).reshape(NC, 128, NC, 128)
    return np.ascontiguousarray(a.transpose(2, 1, 0, 3).reshape(E, E))


def kernel(x, y, Wqkv1, Wqkv2, Wout1, Wout2):
    x = np.asarray(x, dtype=np.float32)
    y = np.asarray(y, dtype=np.float32)

    if "nc" not in _NC_CACHE:
        _NC_CACHE["nc"] = _build()
    nc = _NC_CACHE["nc"]

    # weight prep (host): grouped + transposed (f16 on-device dtype)
    wq1_t = np.ascontiguousarray(_group_w(Wqkv1, 0).T)
    wk1_t = np.ascontiguousarray(_group_w(Wqkv1, 1).T)
    wv1_t = np.ascontiguousarray(_group_w(Wqkv1, 2).T)
    wq2_t = np.ascontiguousarray(_group_w(Wqkv2, 0).T)
    wk2_t = np.ascontiguousarray(_group_w(Wqkv2, 1).T)
    wv2_t = np.ascontiguousarray(_group_w(Wqkv2, 2).T)
    wout1_t = np.ascontiguousarray(np.asarray(Wout1, dtype=np.float32).T)
    wout2_t = np.ascontiguousarray(np.asarray(Wout2, dtype=np.float32).T)

    in_maps = []
    for c in range(N_CORES):
        s, b = divmod(c, B)
        if s == 0:
            # stream-1 output: K,V from x via Wqkv1; Q from y via Wqkv2
            a_t, b_t = x[b].T, y[b].T
            wq, wk, wv, wo = wq2_t, wk1_t, wv1_t, wout1_t
        else:
            a_t, b_t = y[b].T, x[b].T
            wq, wk, wv, wo = wq1_t, wk2_t, wv2_t, wout2_t
        in_maps.append({
            "a_t": np.ascontiguousarray(a_t).astype(np.float16),
            "b_t": np.ascontiguousarray(b_t).astype(np.float16),
            "wq_t": _stream_layout(wq).astype(np.float16),
            "wk_t": _stream_layout(wk).astype(np.float16),
            "wv_t": wv.astype(np.float16),
            "wout_t": _stream_layout(wo).astype(np.float16),
        })

    trace = os.environ.get("BASS_KERNEL_TRACE", "0") == "1"
    if trace:
        try:
            from antenv.axon_hooks import get_axon_ntff_profile_hook  # noqa: F401
        except ImportError:
            trace = False
    ncores = int(os.environ.get("KCORES", str(N_CORES)))
    r = bass_utils.run_bass_kernel_spmd(nc, in_maps[:ncores], core_ids=list(range(ncores)),
                                        trace=trace)
    LAST_RESULTS["exec_time_ns"] = r.exec_time_ns
    LAST_RESULTS["profile_json"] = r.profile_json

    out1 = np.stack([r.results[b]["z_t"].T for b in range(B)]).astype(np.float32)
    out2 = np.stack([r.results[B + b]["z_t"].T for b in range(B)]).astype(np.float32)
    return out1, out2


# revision 32
# speedup vs baseline: 1.5515x; 1.0218x over previous
"""Trainium2 Bass kernel for nn_MultiHeadCrossAttention (B=4, T=1024, E=1024, H=16).

Sharding: 8 fully independent shards (output stream s, batch b), zero
cross-core communication. Stream-1 output xo@Wout1 needs K,V from x and Q
from y; stream-2 the reverse. Core c<4 computes stream-1 batch c; core c>=4
stream-2 batch c-4.

Per-core design: one flat software-pipelined stream over 64 (head-pair m,
key-chunk jc) units, paced by ScalarE exp (the irreducible ~147us of
softmax exponentials). Everything else hides inside that window so the
tensor engine never idles >3.4us (HAM stays at full clock):
  unit (m, jc): S^T = K^T.T @ Q^T (row-paired K=64 MMs, [128,T] PSUM);
                P^T = exp(S^T/8) (2 ACTs); O' ic0-half accumulates
                ([65,512] PSUM, ones-row gives rowsums for free).
  injections (relative to unit stream):
    m=0 units carry V-projection for chunk jc just-in-time.
    (m,0)/(m,1): O' ic1-half of m-1 (A then B, staggered for PSUM slots);
    (m,1)/(m,2): normalize m-1 (reciprocal_approx_fast on free-dim rowsums,
                 K=1 ones-matmul broadcast, DVE mul into O^T f16);
    (m,4)/(m,6): Q^T/K^T projections for chunk m+1 (weights streamed from
                 HBM in host-swizzled contiguous blocks).
  tail: flush m=7, then Z^T = Wout^T.T @ O^T per 128-row chunk (m=7 term
  accumulated last so Z overlaps the flush).
PSUM budget (8 banks): S units 2x[128,1024]=4, O' accums 3x[65,512]=3,
proj/bc scratch 1x[128,512]=1.
Host pre-transposes/groups weights and activations; re-transposes outputs.
"""

import os
import sys

sys.path.insert(0, "/opt/trn_rl_repo")

import numpy as np
import ml_dtypes
from contextlib import ExitStack

import concourse.bass as bass
import concourse.mybir as mybir
import concourse.tile as tile
from concourse import bacc
from concourse import bass_utils

B, T, E, H = 4, 1024, 1024, 16
D = E // H            # 64
NC = E // 128         # 8 chunks of 128
NIC = T // 512        # 2 free-dim chunks of 512
N_CORES = 8

F32 = mybir.dt.float32
F16 = mybir.dt.float16
EXP = mybir.ActivationFunctionType.Exp

_NC_CACHE = {}
LAST_RESULTS = {}


def _build():
    nc = bacc.Bacc("TRN2", target_bir_lowering=False, debug=False,
                   enable_asserts=False, num_devices=N_CORES)
    a_t = nc.dram_tensor("a_t", (E, T), F16, kind="ExternalInput").ap()
    b_t = nc.dram_tensor("b_t", (E, T), F16, kind="ExternalInput").ap()
    wq_t = nc.dram_tensor("wq_t", (E, E), F16, kind="ExternalInput").ap()
    wk_t = nc.dram_tensor("wk_t", (E, E), F16, kind="ExternalInput").ap()
    wv_t = nc.dram_tensor("wv_t", (E, E), F16, kind="ExternalInput").ap()
    wout_t = nc.dram_tensor("wout_t", (E, E), F16, kind="ExternalInput").ap()
    z_t = nc.dram_tensor("z_t", (E, T), F32, kind="ExternalOutput").ap()

    # wq_t/wk_t/wout_t are host-swizzled: block m of w^T is the contiguous
    # [128, 1024] slice rows m*128..(m+1)*128, laid out [p, e*128+c].
    def wblock(w, m):
        return w[m * 128:(m + 1) * 128, :]

    with tile.TileContext(nc) as tc, ExitStack() as ctx:
        persist = ctx.enter_context(tc.tile_pool(name="persist", bufs=1))
        qt = persist.tile([128, NC, T], F16, tag="qt")
        kt = persist.tile([128, NC, T], F16, tag="kt")
        v = persist.tile([128, NC, H * (D + 1)], F16, tag="v")
        at_sb = persist.tile([128, NC, T], F16, tag="at")
        bt_sb = persist.tile([128, NC, T], F16, tag="bt")
        ot = persist.tile([128, NC, T], F16, tag="ot")      # normalized O^T
        ones_t = persist.tile([128, 128], F16, tag="ones")
        nc.vector.memset(ones_t[:], 1.0)

        # DMA order matters: bt + first weight chunks first so Q0/K0 start
        # early; at/wv interleaved so V-proj can chase the transfers.
        wch = ctx.enter_context(tc.tile_pool(name="wch", bufs=2))
        wq0 = wch.tile([128, NC * 128], F16, tag="w", name="wq0")
        wk0 = wch.tile([128, NC * 128], F16, tag="w", name="wk0")
        for c in range(NC):
            nc.sync.dma_start(bt_sb[:, c, :], b_t[c * 128:(c + 1) * 128, :])
        nc.sync.dma_start(wq0[:], wblock(wq_t, 0))
        nc.sync.dma_start(wk0[:], wblock(wk_t, 0))
        for c in range(NC):
            nc.sync.dma_start(at_sb[:, c, :], a_t[c * 128:(c + 1) * 128, :])

        wvp = ctx.enter_context(tc.tile_pool(name="wv", bufs=1))
        wv_sb = wvp.tile([128, NC, E], F16, tag="wv")
        for c in range(NC):
            nc.sync.dma_start(wv_sb[:, c, :], wv_t[c * 128:(c + 1) * 128, :])

        # ones column per head in V (col D within each D+1 group) -> rowsums
        for m in range(NC):
            nc.vector.memset(
                v[:, m, :].rearrange("p (h x) -> p h x", x=D + 1)[:, :, D:D + 1], 1.0)

        ptp = ctx.enter_context(tc.tile_pool(name="pt", bufs=4))
        rsp = ctx.enter_context(tc.tile_pool(name="rsp", bufs=1))
        bcp = ctx.enter_context(tc.tile_pool(name="bcp", bufs=1))
        bigp = ctx.enter_context(tc.tile_pool(name="bigp", bufs=2, space="PSUM"))
        opool = ctx.enter_context(tc.tile_pool(name="op", bufs=4, space="PSUM"))

        def proj_chunk(w_sb, act_sb, out_sb, m, on_act, use_opool=False):
            """One [128, T] projection chunk (contract over e, weight block
            loaded once, both ic-half accumulators live)."""
            if use_opool:
                ps0 = opool.tile([128, 512], F32, tag="o", name=f"pj{m}a")
                ps1 = opool.tile([128, 512], F32, tag="o", name=f"pj{m}b")
            else:
                big = bigp.tile([128, T], F32, tag="big", name=f"pj{m}")
                ps0, ps1 = big[:, 0:512], big[:, 512:1024]
            for e in range(NC):
                st_e = dict(start=(e == 0), stop=(e == NC - 1))
                nc.tensor.matmul(ps0[:, :] if use_opool else ps0,
                                 w_sb[:, bass.ts(e, 128)],
                                 act_sb[:, e, 0:512], **st_e)
                nc.tensor.matmul(ps1[:, :] if use_opool else ps1,
                                 w_sb[:, bass.ts(e, 128)],
                                 act_sb[:, e, 512:1024], **st_e)
            for ic, ps in ((0, ps0), (1, ps1)):
                dst = out_sb[:, m, bass.ts(ic, 512)]
                if on_act:
                    nc.scalar.copy(dst, ps[:, :] if use_opool else ps)
                else:
                    nc.vector.tensor_copy(dst, ps[:, :] if use_opool else ps)

        def vproj_chunk(jc):
            """V chunk jc (natural layout), copy strided into (h, D+1).

            Chunks 0-3 use the (still empty) O'-accumulator slots so they
            don't contend with the S-unit double buffer."""
            if jc < 4:
                ps0 = opool.tile([128, 512], F32, tag="o", name=f"vp{jc}a")
                ps1 = opool.tile([128, 512], F32, tag="o", name=f"vp{jc}b")
                halves = (ps0, ps1)
            else:
                big = bigp.tile([128, T], F32, tag="big", name=f"vps{jc}")
                halves = (big[:, 0:512], big[:, 512:1024])
            for e in range(NC):
                for ic in range(NIC):
                    nc.tensor.matmul(
                        halves[ic], at_sb[:, e, bass.ts(jc, 128)],
                        wv_sb[:, e, bass.ts(ic, 512)],
                        start=(e == 0), stop=(e == NC - 1))
            for ic in range(NIC):
                nc.vector.tensor_copy(
                    v[:, jc, :].rearrange(
                        "p (h x) -> p h x", x=D + 1)[:, ic * 8:(ic + 1) * 8, 0:D],
                    halves[ic].rearrange("p (h x) -> p h x", x=D))

        proj_chunk(wq0, bt_sb, qt, 0, on_act=True)
        proj_chunk(wk0, at_sb, kt, 0, on_act=True)

        # per-m pipeline state
        state = {}

        def normalize_m(m):
            """rowsums -> 1/r -> broadcast -> O^T = O' * (1/r), both ic halves.

            Frees all four of m's O'-accumulator PSUM tiles. The four rowsum
            rows are spread to partitions 0/32/64/96 so the reciprocal runs
            512 elements wide instead of 2048 serial. Its bc matmuls are
            emitted before any PE instruction that waits on those slots.
            """
            st_ = state[m]
            rsf = rsp.tile([128, 512], F32, tag="rsf", name=f"rsf{m}")
            rsr = rsp.tile([128, 512], F32, tag="rsr", name=f"rsr{m}")
            rsr16 = rsp.tile([128, 512], F16, tag="rsr16", name=f"rsr16_{m}")
            rows = {(0, 0): 0, (0, 1): 32, (1, 0): 64, (1, 1): 96}
            for (h, ic), p in rows.items():
                nc.vector.tensor_copy(rsf[p:p + 1, :],
                                      st_[f"acc{ic}_{h}"][64:65, :])
            nc.vector.reciprocal_approx_fast(rsr[:], rsf[:])
            with nc.allow_low_precision(reason="1/r feeds f16 bc matmul"):
                nc.vector.tensor_copy(rsr16[:], rsr[:])
            bc = bigp.tile([128, T], F32, tag="big", name=f"bc{m}")
            for (h, ic), p in rows.items():
                nc.tensor.matmul(
                    bc[h * 64:(h + 1) * 64, bass.ts(ic, 512)],
                    ones_t[p:p + 1, 0:64], rsr16[p:p + 1, :],
                    start=True, stop=True, tile_position=(p, h * 64))
            bcs = bcp.tile([128, T], F32, tag="bcs", name=f"bcs{m}")
            nc.vector.tensor_copy(bcs[:], bc[:])
            with nc.allow_low_precision(reason="O^T f16 feeds f16 out-proj"):
                for ic in range(NIC):
                    s_ic = bass.ts(ic, 512)
                    nc.vector.tensor_mul(ot[0:64, m, s_ic],
                                         st_[f"acc{ic}_0"][0:64, :], bcs[0:64, s_ic])
                    nc.vector.tensor_mul(ot[64:128, m, s_ic],
                                         st_[f"acc{ic}_1"][0:64, :],
                                         bcs[64:128, s_ic])

        # ---------------- the 64-unit stream ----------------
        for u in range(NC * NC):
            m, jc = divmod(u, NC)
            hA, hB = 2 * m, 2 * m + 1
            if jc == 0:
                state[m] = {
                    "ptA": ptp.tile([128, NC, T], F16, tag="pt", name=f"ptA{m}"),
                    "ptB": ptp.tile([128, NC, T], F16, tag="pt", name=f"ptB{m}"),
                }
            st_ = state[m]
            ptA, ptB = st_["ptA"], st_["ptB"]

            ps_s = bigp.tile([128, T], F32, tag="big", name=f"sA{u}")
            ps_sB = bigp.tile([128, T], F32, tag="big", name=f"sB{u}")
            for ic in range(NIC):
                nc.tensor.matmul(
                    ps_s[:, bass.ts(ic, 512)],
                    kt[0:64, m, bass.ts(jc, 128)],
                    qt[0:64, m, bass.ts(ic, 512)],
                    start=True, stop=True)
            for ic in range(NIC):
                nc.tensor.matmul(
                    ps_sB[:, bass.ts(ic, 512)],
                    kt[64:128, m, bass.ts(jc, 128)],
                    qt[64:128, m, bass.ts(ic, 512)],
                    start=True, stop=True, tile_position=(64, 0))
            nc.scalar.activation(ptA[:, jc, :], ps_s[:], EXP, scale=0.125)
            nc.scalar.activation(ptB[:, jc, :], ps_sB[:], EXP, scale=0.125)

            if m == 0:
                vproj_chunk(jc)   # just-in-time V for O' below

            # ---- injections (pipelined work of m-1 / m+1); emitted BEFORE
            # this unit's O' matmuls so normalize's bc matmuls precede any
            # PE instruction that waits on the slots its muls release ----
            if jc == 0 and m >= 1:
                normalize_m(m - 1)
                del state[m - 1]
            elif jc == 1 and m >= 1 and m + 1 < NC:
                wq_sb = wch.tile([128, NC * 128], F16, tag="w", name=f"wq{m+1}")
                nc.sync.dma_start(wq_sb[:], wblock(wq_t, m + 1))
                proj_chunk(wq_sb, bt_sb, qt, m + 1, on_act=False, use_opool=True)
            elif jc == 2 and m >= 1 and m + 1 < NC:
                wk_sb = wch.tile([128, NC * 128], F16, tag="w", name=f"wk{m+1}")
                nc.sync.dma_start(wk_sb[:], wblock(wk_t, m + 1))
                proj_chunk(wk_sb, at_sb, kt, m + 1, on_act=False, use_opool=True)
            elif jc == 4 and m == 0:
                wq_sb = wch.tile([128, NC * 128], F16, tag="w", name="wq1")
                nc.sync.dma_start(wq_sb[:], wblock(wq_t, 1))
                proj_chunk(wq_sb, bt_sb, qt, 1, on_act=False)
            elif jc == 6 and m == 0:
                wk_sb = wch.tile([128, NC * 128], F16, tag="w", name="wk1")
                nc.sync.dma_start(wk_sb[:], wblock(wk_t, 1))
                proj_chunk(wk_sb, at_sb, kt, 1, on_act=False)

            # O' accumulation (both ic halves per weight load). Start
            # deferred to jc==4 so the slot wait lands after proj(m+1) has
            # released the time-shared accumulator slots.
            def o_mms(j2, first):
                stf = dict(start=first, stop=(j2 == NC - 1))
                for ic in range(NIC):
                    nc.tensor.matmul(st_[f"acc{ic}_0"][0:65, :],
                                     v[:, j2, bass.ts(hA, D + 1)],
                                     ptA[:, j2, bass.ts(ic, 512)], **stf)
                for ic in range(NIC):
                    nc.tensor.matmul(st_[f"acc{ic}_1"][0:65, :],
                                     v[:, j2, bass.ts(hB, D + 1)],
                                     ptB[:, j2, bass.ts(ic, 512)], **stf)

            if jc == 4:
                for ic in range(NIC):
                    for h, nm_ in ((0, "A"), (1, "B")):
                        st_[f"acc{ic}_{h}"] = opool.tile(
                            [128, 512], F32, tag="o", name=f"o{ic}{nm_}{m}")
                for j2 in range(5):
                    o_mms(j2, first=(j2 == 0))
            elif jc > 4:
                o_mms(jc, first=False)

        # ---------------- flush m=7 + out-projection ----------------
        normalize_m(NC - 1)

        with tc.tile_pool(name="woch", bufs=3) as wochp, \
             tc.tile_pool(name="zsb", bufs=2) as zsbp:
            for cc in range(NC):
                wo_sb = wochp.tile([128, NC * 128], F16, tag="wo", name=f"wo{cc}")
                nc.sync.dma_start(wo_sb[:], wblock(wout_t, cc))
                ps = bigp.tile([128, T], F32, tag="big", name=f"z{cc}")
                # m=7 term last: its O^T lands during the flush above
                for i, mm in enumerate(range(NC)):
                    for ic in range(NIC):
                        nc.tensor.matmul(
                            ps[:, bass.ts(ic, 512)],
                            wo_sb[:, bass.ts(mm, 128)],
                            ot[:, mm, bass.ts(ic, 512)],
                            start=(i == 0), stop=(i == NC - 1))
                zsb = zsbp.tile([128, T], F32, tag="zsb", name=f"zsb{cc}")
                nc.vector.tensor_copy(zsb[:], ps[:])
                nc.sync.dma_start(z_t[cc * 128:(cc + 1) * 128, :], zsb[:])
    nc.compile()
    return nc


def _group_w(wqkv, k):
    """Rows of Wqkv (3E, E) for q/k/v (k=0/1/2), grouped head-major.

    Row index layout: r = di*(3H) + k*H + h  ->  grouped[h*D+di, :].
    """
    w = np.asarray(wqkv, dtype=np.float32).reshape(D, 3, H, E)[:, k]   # [di, h, e]
    return np.ascontiguousarray(w.transpose(1, 0, 2).reshape(E, E))    # [h*D+di, e]


def _stream_layout(w_t):
    """Swizzle w^T [e*128+p, m*128+c] -> [m*128+p, e*128+c] so the device can
    stream output-block m as one contiguous [128, 1024] DMA."""
    a = np.asarray(w_t).reshape(NC, 128, NC, 128)
    return np.ascontiguousarray(a.transpose(2, 1, 0, 3).reshape(E, E))


def kernel(x, y, Wqkv1, Wqkv2, Wout1, Wout2):
    x = np.asarray(x, dtype=np.float32)
    y = np.asarray(y, dtype=np.float32)

    if "nc" not in _NC_CACHE:
        _NC_CACHE["nc"] = _build()
    nc = _NC_CACHE["nc"]

    # weight prep (host): grouped + transposed (f16 on-device dtype)
    wq1_t = np.ascontiguousarray(_group_w(Wqkv1, 0).T)
    wk1_t = np.ascontiguousarray(_group_w(Wqkv1, 1).T)
    wv1_t = np.ascontiguousarray(_group_w(Wqkv1, 2).T)
    wq2_t = np.ascontiguousarray(_group_w(Wqkv2, 0).T)
    wk2_t = np.ascontiguousarray(_group_w(Wqkv2, 1).T)
    wv2_t = np.ascontiguousarray(_group_w(Wqkv2, 2).T)
    wout1_t = np.ascontiguousarray(np.asarray(Wout1, dtype=np.float32).T)
    wout2_t = np.ascontiguousarray(np.asarray(Wout2, dtype=np.float32).T)

    in_maps = []
    for c in range(N_CORES):
        s, b = divmod(c, B)
        if s == 0:
            # stream-1 output: K,V from x via Wqkv1; Q from y via Wqkv2
            a_t, b_t = x[b].T, y[b].T
            wq, wk, wv, wo = wq2_t, wk1_t, wv1_t, wout1_t
        else:
            a_t, b_t = y[b].T, x[b].T
            wq, wk, wv, wo = wq1_t, wk2_t, wv2_t, wout2_t
        in_maps.append({
            "a_t": np.ascontiguousarray(a_t).astype(np.float16),
            "b_t": np.ascontiguousarray(b_t).astype(np.float16),
            "wq_t": _stream_layout(wq).astype(np.float16),
            "wk_t": _stream_layout(wk).astype(np.float16),
            "wv_t": wv.astype(np.float16),
            "wout_t": _stream_layout(wo).astype(np.float16),
        })

    trace = os.environ.get("BASS_KERNEL_TRACE", "0") == "1"
    if trace:
        try:
            from antenv.axon_hooks import get_axon_ntff_profile_hook  # noqa: F401
        except ImportError:
            trace = False
    ncores = int(os.environ.get("KCORES", str(N_CORES)))
    r = bass_utils.run_bass_kernel_spmd(nc, in_maps[:ncores], core_ids=list(range(ncores)),
                                        trace=trace)
    LAST_RESULTS["exec_time_ns"] = r.exec_time_ns
    LAST_RESULTS["profile_json"] = r.profile_json

    out1 = np.stack([r.results[b]["z_t"].T for b in range(B)]).astype(np.float32)
    out2 = np.stack([r.results[B + b]["z_t"].T for b in range(B)]).astype(np.float32)
    return out1, out2


# revision 34
# speedup vs baseline: 1.5904x; 1.0251x over previous
"""Trainium2 Bass kernel for nn_MultiHeadCrossAttention (B=4, T=1024, E=1024, H=16).

Sharding: 8 fully independent shards (output stream s, batch b), zero
cross-core communication. Stream-1 output xo@Wout1 needs K,V from x and Q
from y; stream-2 the reverse. Core c<4 computes stream-1 batch c; core c>=4
stream-2 batch c-4.

Per-core design: one flat software-pipelined stream over 64 (head-pair m,
key-chunk jc) units. ScalarE exp is the pacing target; all other work is
interleaved so the tensor engine keeps a dense backlog (HAM stays warm):
  unit (m, jc): S^T = K^T.T @ Q^T (row-paired K=64 MMs, [128,T] PSUM,
                one LDWEIGHTS per 2 MMs); P^T = exp(S^T/8) (2 ACTs);
                O' accumulates both ic halves per weight load
                ([128,512] 1-bank PSUM tiles, ones-row gives rowsums free).
  injections (relative to unit stream):
    m=0 units carry V-projection for chunk jc just-in-time (chunks 0-3 on
    the still-empty O' slots, 4-7 on the S pool).
    (m,0): normalize m-1: rowsum rows spread to partitions 0/32/64/96 so
           reciprocal_approx_fast runs 512-wide; K=1 ones-matmul broadcast;
           DVE mul into O^T f16. Frees the four O' accumulator banks.
    (m,1)/(m,2): Q^T/K^T projections for chunk m+1 into the freed O' banks
           (time-shared; weights streamed from HBM in host-swizzled
           contiguous blocks, one LDWEIGHTS per 2 MMs).
    (m,4): O' allocators + catch-up (deferred so the slot wait lands after
           proj released the banks; all waits resolve acyclically vs the
           in-order PE queue - anything a PE instruction waits on is
           produced by instructions emitted earlier).
  tail: normalize m=7, then Z^T = Wout^T.T @ O^T per 128-row chunk (m=7
  term accumulated last so Z overlaps the flush).
PSUM budget (8 banks): S units 2x[128,1024]=4, O'/proj/V time-shared
4x[128,512]=4; bc broadcasts borrow an S slot transiently.
Host pre-transposes/groups weights and activations; re-transposes outputs.
"""

import os
import sys

sys.path.insert(0, "/opt/trn_rl_repo")

import numpy as np
import ml_dtypes
from contextlib import ExitStack

import concourse.bass as bass
import concourse.mybir as mybir
import concourse.tile as tile
from concourse import bacc
from concourse import bass_utils

B, T, E, H = 4, 1024, 1024, 16
D = E // H            # 64
NC = E // 128         # 8 chunks of 128
NIC = T // 512        # 2 free-dim chunks of 512
N_CORES = 8

F32 = mybir.dt.float32
F16 = mybir.dt.float16
EXP = mybir.ActivationFunctionType.Exp

_NC_CACHE = {}
LAST_RESULTS = {}


def _build():
    nc = bacc.Bacc("TRN2", target_bir_lowering=False, debug=False,
                   enable_asserts=False, num_devices=N_CORES)
    a_t = nc.dram_tensor("a_t", (E, T), F16, kind="ExternalInput").ap()
    b_t = nc.dram_tensor("b_t", (E, T), F16, kind="ExternalInput").ap()
    wq_t = nc.dram_tensor("wq_t", (E, E), F16, kind="ExternalInput").ap()
    wk_t = nc.dram_tensor("wk_t", (E, E), F16, kind="ExternalInput").ap()
    wv_t = nc.dram_tensor("wv_t", (E, E), F16, kind="ExternalInput").ap()
    wout_t = nc.dram_tensor("wout_t", (E, E), F16, kind="ExternalInput").ap()
    z_t = nc.dram_tensor("z_t", (E, T), F32, kind="ExternalOutput").ap()

    # wq_t/wk_t/wout_t are host-swizzled: block m of w^T is the contiguous
    # [128, 1024] slice rows m*128..(m+1)*128, laid out [p, e*128+c].
    def wblock(w, m):
        return w[m * 128:(m + 1) * 128, :]

    with tile.TileContext(nc) as tc, ExitStack() as ctx:
        persist = ctx.enter_context(tc.tile_pool(name="persist", bufs=1))
        qt = persist.tile([128, NC, T], F16, tag="qt")
        kt = persist.tile([128, NC, T], F16, tag="kt")
        v = persist.tile([128, NC, H * (D + 1)], F16, tag="v")
        at_sb = persist.tile([128, NC, T], F16, tag="at")
        bt_sb = persist.tile([128, NC, T], F16, tag="bt")
        ot = persist.tile([128, NC, T], F16, tag="ot")      # normalized O^T
        ones_t = persist.tile([128, 128], F16, tag="ones")
        nc.vector.memset(ones_t[:], 1.0)

        # DMA order matters: bt + first weight chunks first so Q0/K0 start
        # early; at/wv interleaved so V-proj can chase the transfers.
        wch = ctx.enter_context(tc.tile_pool(name="wch", bufs=2))
        wq0 = wch.tile([128, NC * 128], F16, tag="w", name="wq0")
        wk0 = wch.tile([128, NC * 128], F16, tag="w", name="wk0")
        for c in range(NC):
            nc.sync.dma_start(bt_sb[:, c, :], b_t[c * 128:(c + 1) * 128, :])
        nc.sync.dma_start(wq0[:], wblock(wq_t, 0))
        nc.sync.dma_start(wk0[:], wblock(wk_t, 0))
        for c in range(NC):
            nc.sync.dma_start(at_sb[:, c, :], a_t[c * 128:(c + 1) * 128, :])

        wvp = ctx.enter_context(tc.tile_pool(name="wv", bufs=1))
        wv_sb = wvp.tile([128, NC, E], F16, tag="wv")
        for c in range(NC):
            nc.sync.dma_start(wv_sb[:, c, :], wv_t[c * 128:(c + 1) * 128, :])

        # ones column per head in V (col D within each D+1 group) -> rowsums
        for m in range(NC):
            nc.vector.memset(
                v[:, m, :].rearrange("p (h x) -> p h x", x=D + 1)[:, :, D:D + 1], 1.0)

        ptp = ctx.enter_context(tc.tile_pool(name="pt", bufs=4))
        rsp = ctx.enter_context(tc.tile_pool(name="rsp", bufs=1))
        bcp = ctx.enter_context(tc.tile_pool(name="bcp", bufs=1))
        bigp = ctx.enter_context(tc.tile_pool(name="bigp", bufs=2, space="PSUM"))
        opool = ctx.enter_context(tc.tile_pool(name="op", bufs=4, space="PSUM"))

        def proj_chunk(w_sb, act_sb, out_sb, m, on_act, use_opool=False):
            """One [128, T] projection chunk (contract over e, weight block
            loaded once, both ic-half accumulators live)."""
            if use_opool:
                ps0 = opool.tile([128, 512], F32, tag="o", name=f"pj{m}a")
                ps1 = opool.tile([128, 512], F32, tag="o", name=f"pj{m}b")
            else:
                big = bigp.tile([128, T], F32, tag="big", name=f"pj{m}")
                ps0, ps1 = big[:, 0:512], big[:, 512:1024]
            for e in range(NC):
                st_e = dict(start=(e == 0), stop=(e == NC - 1))
                nc.tensor.matmul(ps0[:, :] if use_opool else ps0,
                                 w_sb[:, bass.ts(e, 128)],
                                 act_sb[:, e, 0:512], **st_e)
                nc.tensor.matmul(ps1[:, :] if use_opool else ps1,
                                 w_sb[:, bass.ts(e, 128)],
                                 act_sb[:, e, 512:1024], **st_e)
            for ic, ps in ((0, ps0), (1, ps1)):
                dst = out_sb[:, m, bass.ts(ic, 512)]
                if on_act:
                    nc.scalar.copy(dst, ps[:, :] if use_opool else ps)
                else:
                    nc.vector.tensor_copy(dst, ps[:, :] if use_opool else ps)

        def vproj_chunk(jc):
            """V chunk jc (natural layout), copy strided into (h, D+1).

            Chunks 0-3 use the (still empty) O'-accumulator slots so they
            don't contend with the S-unit double buffer."""
            if jc < 4:
                ps0 = opool.tile([128, 512], F32, tag="o", name=f"vp{jc}a")
                ps1 = opool.tile([128, 512], F32, tag="o", name=f"vp{jc}b")
                halves = (ps0, ps1)
            else:
                big = bigp.tile([128, T], F32, tag="big", name=f"vps{jc}")
                halves = (big[:, 0:512], big[:, 512:1024])
            for e in range(NC):
                for ic in range(NIC):
                    nc.tensor.matmul(
                        halves[ic], at_sb[:, e, bass.ts(jc, 128)],
                        wv_sb[:, e, bass.ts(ic, 512)],
                        start=(e == 0), stop=(e == NC - 1))
            for ic in range(NIC):
                nc.vector.tensor_copy(
                    v[:, jc, :].rearrange(
                        "p (h x) -> p h x", x=D + 1)[:, ic * 8:(ic + 1) * 8, 0:D],
                    halves[ic].rearrange("p (h x) -> p h x", x=D))

        proj_chunk(wq0, bt_sb, qt, 0, on_act=True)
        proj_chunk(wk0, at_sb, kt, 0, on_act=True)

        # per-m pipeline state
        state = {}

        NORM_ROWS = {(0, 0): 0, (0, 1): 32, (1, 0): 64, (1, 1): 96}

        def normalize_pre(m):
            """DVE-only prefix: rowsums -> 1/r (f16). The four rowsum rows
            are spread to partitions 0/32/64/96 so the reciprocal runs 512
            elements wide instead of 2048 serial."""
            st_ = state[m]
            rsf = rsp.tile([128, 512], F32, tag="rsf", name=f"rsf{m}")
            rsr = rsp.tile([128, 512], F32, tag="rsr", name=f"rsr{m}")
            rsr16 = rsp.tile([128, 512], F16, tag="rsr16", name=f"rsr16_{m}")
            for (h, ic), p in NORM_ROWS.items():
                nc.vector.tensor_copy(rsf[p:p + 1, :],
                                      st_[f"acc{ic}_{h}"][64:65, :])
            nc.vector.reciprocal_approx_fast(rsr[:], rsf[:])
            with nc.allow_low_precision(reason="1/r feeds f16 bc matmul"):
                nc.vector.tensor_copy(rsr16[:], rsr[:])
            st_["rsr16"] = rsr16

        def normalize_post(m):
            """Broadcast (K=1 ones-matmul) + DVE mul into O^T f16. Frees all
            four of m's O'-accumulator PSUM tiles; its bc matmuls precede
            any PE instruction waiting on those slots."""
            st_ = state[m]
            rsr16 = st_["rsr16"]
            bc = bigp.tile([128, T], F32, tag="big", name=f"bc{m}")
            for (h, ic), p in NORM_ROWS.items():
                nc.tensor.matmul(
                    bc[h * 64:(h + 1) * 64, bass.ts(ic, 512)],
                    ones_t[p:p + 1, 0:64], rsr16[p:p + 1, :],
                    start=True, stop=True, tile_position=(p, h * 64))
            bcs = bcp.tile([128, T], F32, tag="bcs", name=f"bcs{m}")
            nc.vector.tensor_copy(bcs[:], bc[:])
            with nc.allow_low_precision(reason="O^T f16 feeds f16 out-proj"):
                for ic in range(NIC):
                    s_ic = bass.ts(ic, 512)
                    nc.vector.tensor_mul(ot[0:64, m, s_ic],
                                         st_[f"acc{ic}_0"][0:64, :], bcs[0:64, s_ic])
                    nc.vector.tensor_mul(ot[64:128, m, s_ic],
                                         st_[f"acc{ic}_1"][0:64, :],
                                         bcs[64:128, s_ic])

        # ---------------- the 64-unit stream ----------------
        for u in range(NC * NC):
            m, jc = divmod(u, NC)
            hA, hB = 2 * m, 2 * m + 1
            if jc == 0:
                state[m] = {
                    "ptA": ptp.tile([128, NC, T], F16, tag="pt", name=f"ptA{m}"),
                    "ptB": ptp.tile([128, NC, T], F16, tag="pt", name=f"ptB{m}"),
                }
            st_ = state[m]
            ptA, ptB = st_["ptA"], st_["ptB"]

            ps_s = bigp.tile([128, T], F32, tag="big", name=f"sA{u}")
            ps_sB = bigp.tile([128, T], F32, tag="big", name=f"sB{u}")
            for ic in range(NIC):
                nc.tensor.matmul(
                    ps_s[:, bass.ts(ic, 512)],
                    kt[0:64, m, bass.ts(jc, 128)],
                    qt[0:64, m, bass.ts(ic, 512)],
                    start=True, stop=True)
            for ic in range(NIC):
                nc.tensor.matmul(
                    ps_sB[:, bass.ts(ic, 512)],
                    kt[64:128, m, bass.ts(jc, 128)],
                    qt[64:128, m, bass.ts(ic, 512)],
                    start=True, stop=True, tile_position=(64, 0))
            nc.scalar.activation(ptA[:, jc, :], ps_s[:], EXP, scale=0.125)
            nc.scalar.activation(ptB[:, jc, :], ps_sB[:], EXP, scale=0.125)

            if m == 0:
                vproj_chunk(jc)   # just-in-time V for O' below

            # ---- injections (pipelined work of m-1 / m+1); emitted BEFORE
            # this unit's O' matmuls so normalize's bc matmuls precede any
            # PE instruction that waits on the slots its muls release ----
            if jc == 0 and m >= 1:
                normalize_pre(m - 1)
            elif jc == 1 and m >= 1:
                normalize_post(m - 1)
                del state[m - 1]
            elif jc == 2 and m >= 1 and m + 1 < NC:
                wq_sb = wch.tile([128, NC * 128], F16, tag="w", name=f"wq{m+1}")
                nc.sync.dma_start(wq_sb[:], wblock(wq_t, m + 1))
                proj_chunk(wq_sb, bt_sb, qt, m + 1, on_act=False, use_opool=True)
            elif jc == 3 and m >= 1 and m + 1 < NC:
                wk_sb = wch.tile([128, NC * 128], F16, tag="w", name=f"wk{m+1}")
                nc.sync.dma_start(wk_sb[:], wblock(wk_t, m + 1))
                proj_chunk(wk_sb, at_sb, kt, m + 1, on_act=False, use_opool=True)
            elif jc == 4 and m == 0:
                wq_sb = wch.tile([128, NC * 128], F16, tag="w", name="wq1")
                nc.sync.dma_start(wq_sb[:], wblock(wq_t, 1))
                proj_chunk(wq_sb, bt_sb, qt, 1, on_act=False)
            elif jc == 6 and m == 0:
                wk_sb = wch.tile([128, NC * 128], F16, tag="w", name="wk1")
                nc.sync.dma_start(wk_sb[:], wblock(wk_t, 1))
                proj_chunk(wk_sb, at_sb, kt, 1, on_act=False)

            # O' accumulation (both ic halves per weight load). Start
            # deferred to jc==4 so the slot wait lands after proj(m+1) has
            # released the time-shared accumulator slots.
            def o_mms(j2, first):
                stf = dict(start=first, stop=(j2 == NC - 1))
                for ic in range(NIC):
                    nc.tensor.matmul(st_[f"acc{ic}_0"][0:65, :],
                                     v[:, j2, bass.ts(hA, D + 1)],
                                     ptA[:, j2, bass.ts(ic, 512)], **stf)
                for ic in range(NIC):
                    nc.tensor.matmul(st_[f"acc{ic}_1"][0:65, :],
                                     v[:, j2, bass.ts(hB, D + 1)],
                                     ptB[:, j2, bass.ts(ic, 512)], **stf)

            if jc == 4:
                for ic in range(NIC):
                    for h, nm_ in ((0, "A"), (1, "B")):
                        st_[f"acc{ic}_{h}"] = opool.tile(
                            [128, 512], F32, tag="o", name=f"o{ic}{nm_}{m}")
                for j2 in range(5):
                    o_mms(j2, first=(j2 == 0))
            elif jc > 4:
                o_mms(jc, first=False)

        # ---------------- flush m=7 + out-projection ----------------
        normalize_pre(NC - 1)
        normalize_post(NC - 1)

        with tc.tile_pool(name="woch", bufs=3) as wochp, \
             tc.tile_pool(name="zsb", bufs=2) as zsbp:
            for cc in range(NC):
                wo_sb = wochp.tile([128, NC * 128], F16, tag="wo", name=f"wo{cc}")
                nc.sync.dma_start(wo_sb[:], wblock(wout_t, cc))
                ps = bigp.tile([128, T], F32, tag="big", name=f"z{cc}")
                # m=7 term last: its O^T lands during the flush above
                for i, mm in enumerate(range(NC)):
                    for ic in range(NIC):
                        nc.tensor.matmul(
                            ps[:, bass.ts(ic, 512)],
                            wo_sb[:, bass.ts(mm, 128)],
                            ot[:, mm, bass.ts(ic, 512)],
                            start=(i == 0), stop=(i == NC - 1))
                zsb = zsbp.tile([128, T], F32, tag="zsb", name=f"zsb{cc}")
                nc.vector.tensor_copy(zsb[:], ps[:])
                nc.sync.dma_start(z_t[cc * 128:(cc + 1) * 128, :], zsb[:])
    nc.compile()
    return nc


def _group_w(wqkv, k):
    """Rows of Wqkv (3E, E) for q/k/v (k=0/1/2), grouped head-major.

    Row index layout: r = di*(3H) + k*H + h  ->  grouped[h*D+di, :].
    """
    w = np.asarray(wqkv, dtype=np.float32).reshape(D, 3, H, E)[:, k]   # [di, h, e]
    return np.ascontiguousarray(w.transpose(1, 0, 2).reshape(E, E))    # [h*D+di, e]


def _stream_layout(w_t):
    """Swizzle w^T [e*128+p, m*128+c] -> [m*128+p, e*128+c] so the device can
    stream output-block m as one contiguous [128, 1024] DMA."""
    a = np.asarray(w_t).reshape(NC, 128, NC, 128)
    return np.ascontiguousarray(a.transpose(2, 1, 0, 3).reshape(E, E))


def kernel(x, y, Wqkv1, Wqkv2, Wout1, Wout2):
    x = np.asarray(x, dtype=np.float32)
    y = np.asarray(y, dtype=np.float32)

    if "nc" not in _NC_CACHE:
        _NC_CACHE["nc"] = _build()
    nc = _NC_CACHE["nc"]

    # weight prep (host): grouped + transposed (f16 on-device dtype)
    wq1_t = np.ascontiguousarray(_group_w(Wqkv1, 0).T)
    wk1_t = np.ascontiguousarray(_group_w(Wqkv1, 1).T)
    wv1_t = np.ascontiguousarray(_group_w(Wqkv1, 2).T)
    wq2_t = np.ascontiguousarray(_group_w(Wqkv2, 0).T)
    wk2_t = np.ascontiguousarray(_group_w(Wqkv2, 1).T)
    wv2_t = np.ascontiguousarray(_group_w(Wqkv2, 2).T)
    wout1_t = np.ascontiguousarray(np.asarray(Wout1, dtype=np.float32).T)
    wout2_t = np.ascontiguousarray(np.asarray(Wout2, dtype=np.float32).T)

    in_maps = []
    for c in range(N_CORES):
        s, b = divmod(c, B)
        if s == 0:
            # stream-1 output: K,V from x via Wqkv1; Q from y via Wqkv2
            a_t, b_t = x[b].T, y[b].T
            wq, wk, wv, wo = wq2_t, wk1_t, wv1_t, wout1_t
        else:
            a_t, b_t = y[b].T, x[b].T
            wq, wk, wv, wo = wq1_t, wk2_t, wv2_t, wout2_t
        in_maps.append({
            "a_t": np.ascontiguousarray(a_t).astype(np.float16),
            "b_t": np.ascontiguousarray(b_t).astype(np.float16),
            "wq_t": _stream_layout(wq).astype(np.float16),
            "wk_t": _stream_layout(wk).astype(np.float16),
            "wv_t": wv.astype(np.float16),
            "wout_t": _stream_layout(wo).astype(np.float16),
        })

    trace = os.environ.get("BASS_KERNEL_TRACE", "0") == "1"
    if trace:
        try:
            from antenv.axon_hooks import get_axon_ntff_profile_hook  # noqa: F401
        except ImportError:
            trace = False
    ncores = int(os.environ.get("KCORES", str(N_CORES)))
    r = bass_utils.run_bass_kernel_spmd(nc, in_maps[:ncores], core_ids=list(range(ncores)),
                                        trace=trace)
    LAST_RESULTS["exec_time_ns"] = r.exec_time_ns
    LAST_RESULTS["profile_json"] = r.profile_json

    out1 = np.stack([r.results[b]["z_t"].T for b in range(B)]).astype(np.float32)
    out2 = np.stack([r.results[B + b]["z_t"].T for b in range(B)]).astype(np.float32)
    return out1, out2


# revision 35
# speedup vs baseline: 1.6169x; 1.0166x over previous
"""Trainium2 Bass kernel for nn_MultiHeadCrossAttention (B=4, T=1024, E=1024, H=16).

Sharding: 8 fully independent shards (output stream s, batch b), zero
cross-core communication. Stream-1 output xo@Wout1 needs K,V from x and Q
from y; stream-2 the reverse. Core c<4 computes stream-1 batch c; core c>=4
stream-2 batch c-4.

Per-core design: one flat software-pipelined stream over 64 (head-pair m,
key-chunk jc) units. ScalarE exp is the pacing target; all other work is
interleaved so the tensor engine keeps a dense backlog (HAM stays warm):
  unit (m, jc): S^T = K^T.T @ Q^T (row-paired K=64 MMs, [128,T] PSUM,
                one LDWEIGHTS per 2 MMs); P^T = exp(S^T/8) (2 ACTs);
                O' accumulates both ic halves per weight load
                ([128,512] 1-bank PSUM tiles, ones-row gives rowsums free).
  injections (relative to unit stream):
    m=0 units carry V-projection for chunk jc just-in-time (chunks 0-3 on
    the still-empty O' slots, 4-7 on the S pool).
    (m,0): normalize m-1: rowsum rows spread to partitions 0/32/64/96 so
           reciprocal_approx_fast runs 512-wide; K=1 ones-matmul broadcast;
           DVE mul into O^T f16. Frees the four O' accumulator banks.
    (m,1)/(m,2): Q^T/K^T projections for chunk m+1 into the freed O' banks
           (time-shared; weights streamed from HBM in host-swizzled
           contiguous blocks, one LDWEIGHTS per 2 MMs).
    (m,4): O' allocators + catch-up (deferred so the slot wait lands after
           proj released the banks; all waits resolve acyclically vs the
           in-order PE queue - anything a PE instruction waits on is
           produced by instructions emitted earlier).
  tail: normalize m=7, then Z^T = Wout^T.T @ O^T per 128-row chunk (m=7
  term accumulated last so Z overlaps the flush).
PSUM budget (8 banks): S units 2x[128,1024]=4, O'/proj/V time-shared
4x[128,512]=4; bc broadcasts borrow an S slot transiently.
Host pre-transposes/groups weights and activations; re-transposes outputs.
"""

import os
import sys

sys.path.insert(0, "/opt/trn_rl_repo")

import numpy as np
import ml_dtypes
from contextlib import ExitStack

import concourse.bass as bass
import concourse.mybir as mybir
import concourse.tile as tile
from concourse import bacc
from concourse import bass_utils

B, T, E, H = 4, 1024, 1024, 16
D = E // H            # 64
NC = E // 128         # 8 chunks of 128
NIC = T // 512        # 2 free-dim chunks of 512
N_CORES = 8

F32 = mybir.dt.float32
F16 = mybir.dt.float16
EXP = mybir.ActivationFunctionType.Exp

_NC_CACHE = {}
LAST_RESULTS = {}


def _build():
    nc = bacc.Bacc("TRN2", target_bir_lowering=False, debug=False,
                   enable_asserts=False, num_devices=N_CORES)
    a_t = nc.dram_tensor("a_t", (E, T), F16, kind="ExternalInput").ap()
    b_t = nc.dram_tensor("b_t", (E, T), F16, kind="ExternalInput").ap()
    wq_t = nc.dram_tensor("wq_t", (E, E), F16, kind="ExternalInput").ap()
    wk_t = nc.dram_tensor("wk_t", (E, E), F16, kind="ExternalInput").ap()
    wv_t = nc.dram_tensor("wv_t", (E, E), F16, kind="ExternalInput").ap()
    wout_t = nc.dram_tensor("wout_t", (E, E), F16, kind="ExternalInput").ap()
    z_t = nc.dram_tensor("z_t", (E, T), F32, kind="ExternalOutput").ap()

    # wq_t/wk_t/wout_t are host-swizzled: block m of w^T is the contiguous
    # [128, 1024] slice rows m*128..(m+1)*128, laid out [p, e*128+c].
    def wblock(w, m):
        return w[m * 128:(m + 1) * 128, :]

    with tile.TileContext(nc) as tc, ExitStack() as ctx:
        persist = ctx.enter_context(tc.tile_pool(name="persist", bufs=1))
        qt = persist.tile([128, NC, T], F16, tag="qt")
        kt = persist.tile([128, NC, T], F16, tag="kt")
        v = persist.tile([128, NC, H * (D + 1)], F16, tag="v")
        at_sb = persist.tile([128, NC, T], F16, tag="at")
        bt_sb = persist.tile([128, NC, T], F16, tag="bt")
        ot = persist.tile([128, NC, T], F16, tag="ot")      # normalized O^T
        ones_t = persist.tile([128, 128], F16, tag="ones")
        nc.vector.memset(ones_t[:], 1.0)

        # DMA order matters: bt + first weight chunks first so Q0/K0 start
        # early; at/wv interleaved so V-proj can chase the transfers.
        wch = ctx.enter_context(tc.tile_pool(name="wch", bufs=2))
        wq0 = wch.tile([128, NC * 128], F16, tag="w", name="wq0")
        wk0 = wch.tile([128, NC * 128], F16, tag="w", name="wk0")
        for c in range(NC):
            nc.sync.dma_start(bt_sb[:, c, :], b_t[c * 128:(c + 1) * 128, :])
        nc.sync.dma_start(wq0[:], wblock(wq_t, 0))
        nc.sync.dma_start(wk0[:], wblock(wk_t, 0))
        for c in range(NC):
            nc.sync.dma_start(at_sb[:, c, :], a_t[c * 128:(c + 1) * 128, :])

        wvp = ctx.enter_context(tc.tile_pool(name="wv", bufs=1))
        wv_sb = wvp.tile([128, NC, E], F16, tag="wv")
        for c in range(NC):
            nc.sync.dma_start(wv_sb[:, c, :], wv_t[c * 128:(c + 1) * 128, :])

        # ones column per head in V (col D within each D+1 group) -> rowsums
        for m in range(NC):
            nc.vector.memset(
                v[:, m, :].rearrange("p (h x) -> p h x", x=D + 1)[:, :, D:D + 1], 1.0)

        ptp = ctx.enter_context(tc.tile_pool(name="pt", bufs=4))
        rsp = ctx.enter_context(tc.tile_pool(name="rsp", bufs=1))
        bcp = ctx.enter_context(tc.tile_pool(name="bcp", bufs=1))
        bigp = ctx.enter_context(tc.tile_pool(name="bigp", bufs=2, space="PSUM"))
        opool = ctx.enter_context(tc.tile_pool(name="op", bufs=4, space="PSUM"))

        def proj_chunk(w_sb, act_sb, out_sb, m, on_act, use_opool=False):
            """One [128, T] projection chunk (contract over e, weight block
            loaded once, both ic-half accumulators live)."""
            if use_opool:
                ps0 = opool.tile([128, 512], F32, tag="o", name=f"pj{m}a")
                ps1 = opool.tile([128, 512], F32, tag="o", name=f"pj{m}b")
            else:
                big = bigp.tile([128, T], F32, tag="big", name=f"pj{m}")
                ps0, ps1 = big[:, 0:512], big[:, 512:1024]
            for e in range(NC):
                st_e = dict(start=(e == 0), stop=(e == NC - 1))
                nc.tensor.matmul(ps0[:, :] if use_opool else ps0,
                                 w_sb[:, bass.ts(e, 128)],
                                 act_sb[:, e, 0:512], **st_e)
                nc.tensor.matmul(ps1[:, :] if use_opool else ps1,
                                 w_sb[:, bass.ts(e, 128)],
                                 act_sb[:, e, 512:1024], **st_e)
            for ic, ps in ((0, ps0), (1, ps1)):
                dst = out_sb[:, m, bass.ts(ic, 512)]
                if on_act:
                    nc.scalar.copy(dst, ps[:, :] if use_opool else ps)
                else:
                    nc.vector.tensor_copy(dst, ps[:, :] if use_opool else ps)

        def vproj_chunk(jc):
            """V chunk jc (natural layout), copy strided into (h, D+1).

            Chunks 0-3 use the (still empty) O'-accumulator slots so they
            don't contend with the S-unit double buffer."""
            if jc < 4:
                ps0 = opool.tile([128, 512], F32, tag="o", name=f"vp{jc}a")
                ps1 = opool.tile([128, 512], F32, tag="o", name=f"vp{jc}b")
                halves = (ps0, ps1)
            else:
                big = bigp.tile([128, T], F32, tag="big", name=f"vps{jc}")
                halves = (big[:, 0:512], big[:, 512:1024])
            for e in range(NC):
                for ic in range(NIC):
                    nc.tensor.matmul(
                        halves[ic], at_sb[:, e, bass.ts(jc, 128)],
                        wv_sb[:, e, bass.ts(ic, 512)],
                        start=(e == 0), stop=(e == NC - 1))
            for ic in range(NIC):
                nc.vector.tensor_copy(
                    v[:, jc, :].rearrange(
                        "p (h x) -> p h x", x=D + 1)[:, ic * 8:(ic + 1) * 8, 0:D],
                    halves[ic].rearrange("p (h x) -> p h x", x=D))

        # HAM warm-up: dense junk matmuls on the first bt chunk while the
        # remaining input DMAs stream, so Q0/K0 run at full clock
        warm = bigp.tile([128, T], F32, tag="big", name="warm")
        for _ in range(16):
            nc.tensor.matmul(warm[:, 0:512], bt_sb[:, 0, 0:128],
                             bt_sb[:, 0, 0:512], start=True, stop=True)

        proj_chunk(wq0, bt_sb, qt, 0, on_act=True)
        proj_chunk(wk0, at_sb, kt, 0, on_act=True)

        # per-m pipeline state
        state = {}

        NORM_ROWS = {(0, 0): 0, (0, 1): 32, (1, 0): 64, (1, 1): 96}

        def normalize_pre(m):
            """DVE-only prefix: rowsums -> 1/r (f16). The four rowsum rows
            are spread to partitions 0/32/64/96 so the reciprocal runs 512
            elements wide instead of 2048 serial."""
            st_ = state[m]
            rsf = rsp.tile([128, 512], F32, tag="rsf", name=f"rsf{m}")
            rsr = rsp.tile([128, 512], F32, tag="rsr", name=f"rsr{m}")
            rsr16 = rsp.tile([128, 512], F16, tag="rsr16", name=f"rsr16_{m}")
            for (h, ic), p in NORM_ROWS.items():
                nc.vector.tensor_copy(rsf[p:p + 1, :],
                                      st_[f"acc{ic}_{h}"][64:65, :])
            nc.vector.reciprocal_approx_fast(rsr[:], rsf[:])
            with nc.allow_low_precision(reason="1/r feeds f16 bc matmul"):
                nc.vector.tensor_copy(rsr16[:], rsr[:])
            st_["rsr16"] = rsr16

        def normalize_post(m):
            """Broadcast (K=1 ones-matmul) + DVE mul into O^T f16. Frees all
            four of m's O'-accumulator PSUM tiles; its bc matmuls precede
            any PE instruction waiting on those slots."""
            st_ = state[m]
            rsr16 = st_["rsr16"]
            bc = bigp.tile([128, T], F32, tag="big", name=f"bc{m}")
            for (h, ic), p in NORM_ROWS.items():
                nc.tensor.matmul(
                    bc[h * 64:(h + 1) * 64, bass.ts(ic, 512)],
                    ones_t[p:p + 1, 0:64], rsr16[p:p + 1, :],
                    start=True, stop=True, tile_position=(p, h * 64))
            bcs = bcp.tile([128, T], F32, tag="bcs", name=f"bcs{m}")
            nc.vector.tensor_copy(bcs[:], bc[:])
            with nc.allow_low_precision(reason="O^T f16 feeds f16 out-proj"):
                for ic in range(NIC):
                    s_ic = bass.ts(ic, 512)
                    nc.vector.tensor_mul(ot[0:64, m, s_ic],
                                         st_[f"acc{ic}_0"][0:64, :], bcs[0:64, s_ic])
                    nc.vector.tensor_mul(ot[64:128, m, s_ic],
                                         st_[f"acc{ic}_1"][0:64, :],
                                         bcs[64:128, s_ic])

        # ---------------- the 64-unit stream ----------------
        for u in range(NC * NC):
            m, jc = divmod(u, NC)
            hA, hB = 2 * m, 2 * m + 1
            if jc == 0:
                state[m] = {
                    "ptA": ptp.tile([128, NC, T], F16, tag="pt", name=f"ptA{m}"),
                    "ptB": ptp.tile([128, NC, T], F16, tag="pt", name=f"ptB{m}"),
                }
            st_ = state[m]
            ptA, ptB = st_["ptA"], st_["ptB"]

            ps_s = bigp.tile([128, T], F32, tag="big", name=f"sA{u}")
            ps_sB = bigp.tile([128, T], F32, tag="big", name=f"sB{u}")
            for ic in range(NIC):
                nc.tensor.matmul(
                    ps_s[:, bass.ts(ic, 512)],
                    kt[0:64, m, bass.ts(jc, 128)],
                    qt[0:64, m, bass.ts(ic, 512)],
                    start=True, stop=True)
            for ic in range(NIC):
                nc.tensor.matmul(
                    ps_sB[:, bass.ts(ic, 512)],
                    kt[64:128, m, bass.ts(jc, 128)],
                    qt[64:128, m, bass.ts(ic, 512)],
                    start=True, stop=True, tile_position=(64, 0))
            nc.scalar.activation(ptA[:, jc, :], ps_s[:], EXP, scale=0.125)
            nc.scalar.activation(ptB[:, jc, :], ps_sB[:], EXP, scale=0.125)

            if m == 0:
                vproj_chunk(jc)   # just-in-time V for O' below

            # ---- injections (pipelined work of m-1 / m+1); emitted BEFORE
            # this unit's O' matmuls so normalize's bc matmuls precede any
            # PE instruction that waits on the slots its muls release ----
            if jc == 0 and m >= 1:
                normalize_pre(m - 1)
            elif jc == 1 and m >= 1:
                normalize_post(m - 1)
                del state[m - 1]
            elif jc == 2 and m >= 1 and m + 1 < NC:
                wq_sb = wch.tile([128, NC * 128], F16, tag="w", name=f"wq{m+1}")
                nc.sync.dma_start(wq_sb[:], wblock(wq_t, m + 1))
                proj_chunk(wq_sb, bt_sb, qt, m + 1, on_act=False, use_opool=True)
            elif jc == 3 and m >= 1 and m + 1 < NC:
                wk_sb = wch.tile([128, NC * 128], F16, tag="w", name=f"wk{m+1}")
                nc.sync.dma_start(wk_sb[:], wblock(wk_t, m + 1))
                proj_chunk(wk_sb, at_sb, kt, m + 1, on_act=False, use_opool=True)
            elif jc == 4 and m == 0:
                wq_sb = wch.tile([128, NC * 128], F16, tag="w", name="wq1")
                nc.sync.dma_start(wq_sb[:], wblock(wq_t, 1))
                proj_chunk(wq_sb, bt_sb, qt, 1, on_act=False)
            elif jc == 6 and m == 0:
                wk_sb = wch.tile([128, NC * 128], F16, tag="w", name="wk1")
                nc.sync.dma_start(wk_sb[:], wblock(wk_t, 1))
                proj_chunk(wk_sb, at_sb, kt, 1, on_act=False)

            # O' accumulation (both ic halves per weight load). Start
            # deferred to jc==4 so the slot wait lands after proj(m+1) has
            # released the time-shared accumulator slots.
            def o_mms(j2, first):
                stf = dict(start=first, stop=(j2 == NC - 1))
                for ic in range(NIC):
                    nc.tensor.matmul(st_[f"acc{ic}_0"][0:65, :],
                                     v[:, j2, bass.ts(hA, D + 1)],
                                     ptA[:, j2, bass.ts(ic, 512)], **stf)
                for ic in range(NIC):
                    nc.tensor.matmul(st_[f"acc{ic}_1"][0:65, :],
                                     v[:, j2, bass.ts(hB, D + 1)],
                                     ptB[:, j2, bass.ts(ic, 512)], **stf)

            if jc == 4:
                for ic in range(NIC):
                    for h, nm_ in ((0, "A"), (1, "B")):
                        st_[f"acc{ic}_{h}"] = opool.tile(
                            [128, 512], F32, tag="o", name=f"o{ic}{nm_}{m}")
                for j2 in range(3):
                    o_mms(j2, first=(j2 == 0))
            elif jc == 5:
                for j2 in (3, 4, 5):
                    o_mms(j2, first=False)
            elif jc > 5:
                o_mms(jc, first=False)

        # ---------------- flush m=7 + out-projection ----------------
        normalize_pre(NC - 1)
        normalize_post(NC - 1)

        with tc.tile_pool(name="woch", bufs=3) as wochp, \
             tc.tile_pool(name="zsb", bufs=2) as zsbp:
            for cc in range(NC):
                wo_sb = wochp.tile([128, NC * 128], F16, tag="wo", name=f"wo{cc}")
                nc.sync.dma_start(wo_sb[:], wblock(wout_t, cc))
                ps = bigp.tile([128, T], F32, tag="big", name=f"z{cc}")
                # m=7 term last: its O^T lands during the flush above
                for i, mm in enumerate(range(NC)):
                    for ic in range(NIC):
                        nc.tensor.matmul(
                            ps[:, bass.ts(ic, 512)],
                            wo_sb[:, bass.ts(mm, 128)],
                            ot[:, mm, bass.ts(ic, 512)],
                            start=(i == 0), stop=(i == NC - 1))
                zsb = zsbp.tile([128, T], F32, tag="zsb", name=f"zsb{cc}")
                nc.vector.tensor_copy(zsb[:], ps[:])
                nc.sync.dma_start(z_t[cc * 128:(cc + 1) * 128, :], zsb[:])
    nc.compile()
    return nc


def _group_w(wqkv, k):
    """Rows of Wqkv (3E, E) for q/k/v (k=0/1/2), grouped head-major.

    Row index layout: r = di*(3H) + k*H + h  ->  grouped[h*D+di, :].
    """
    w = np.asarray(wqkv, dtype=np.float32).reshape(D, 3, H, E)[:, k]   # [di, h, e]
    return np.ascontiguousarray(w.transpose(1, 0, 2).reshape(E, E))    # [h*D+di, e]


def _stream_layout(w_t):
    """Swizzle w^T [e*128+p, m*128+c] -> [m*128+p, e*128+c] so the device can
    stream output-block m as one contiguous [128, 1024] DMA."""
    a = np.asarray(w_t).reshape(NC, 128, NC, 128)
    return np.ascontiguousarray(a.transpose(2, 1, 0, 3).reshape(E, E))


def kernel(x, y, Wqkv1, Wqkv2, Wout1, Wout2):
    x = np.asarray(x, dtype=np.float32)
    y = np.asarray(y, dtype=np.float32)

    if "nc" not in _NC_CACHE:
        _NC_CACHE["nc"] = _build()
    nc = _NC_CACHE["nc"]

    # weight prep (host): grouped + transposed (f16 on-device dtype)
    wq1_t = np.ascontiguousarray(_group_w(Wqkv1, 0).T)
    wk1_t = np.ascontiguousarray(_group_w(Wqkv1, 1).T)
    wv1_t = np.ascontiguousarray(_group_w(Wqkv1, 2).T)
    wq2_t = np.ascontiguousarray(_group_w(Wqkv2, 0).T)
    wk2_t = np.ascontiguousarray(_group_w(Wqkv2, 1).T)
    wv2_t = np.ascontiguousarray(_group_w(Wqkv2, 2).T)
    wout1_t = np.ascontiguousarray(np.asarray(Wout1, dtype=np.float32).T)
    wout2_t = np.ascontiguousarray(np.asarray(Wout2, dtype=np.float32).T)

    in_maps = []
    for c in range(N_CORES):
        s, b = divmod(c, B)
        if s == 0:
            # stream-1 output: K,V from x via Wqkv1; Q from y via Wqkv2
            a_t, b_t = x[b].T, y[b].T
            wq, wk, wv, wo = wq2_t, wk1_t, wv1_t, wout1_t
        else:
            a_t, b_t = y[b].T, x[b].T
            wq, wk, wv, wo = wq1_t, wk2_t, wv2_t, wout2_t
        in_maps.append({
            "a_t": np.ascontiguousarray(a_t).astype(np.float16),
            "b_t": np.ascontiguousarray(b_t).astype(np.float16),
            "wq_t": _stream_layout(wq).astype(np.float16),
            "wk_t": _stream_layout(wk).astype(np.float16),
            "wv_t": wv.astype(np.float16),
            "wout_t": _stream_layout(wo).astype(np.float16),
        })

    trace = os.environ.get("BASS_KERNEL_TRACE", "0") == "1"
    if trace:
        try:
            from antenv.axon_hooks import get_axon_ntff_profile_hook  # noqa: F401
        except ImportError:
            trace = False
    ncores = int(os.environ.get("KCORES", str(N_CORES)))
    r = bass_utils.run_bass_kernel_spmd(nc, in_maps[:ncores], core_ids=list(range(ncores)),
                                        trace=trace)
    LAST_RESULTS["exec_time_ns"] = r.exec_time_ns
    LAST_RESULTS["profile_json"] = r.profile_json

    out1 = np.stack([r.results[b]["z_t"].T for b in range(B)]).astype(np.float32)
    out2 = np.stack([r.results[B + b]["z_t"].T for b in range(B)]).astype(np.float32)
    return out1, out2
